# revision 13
# baseline (speedup 1.0000x reference)
"""Trainium2 Bass kernel for nn_BaseX2HAttLayer (GNN edge-softmax attention).

v2 strategy (per core, edges sorted by dst into 10 buckets of 128 dst nodes):
  - src features enter the kv MLP via a TRANSPOSED batched dma_gather
    (hsT[d, e] = h[src_e, d]) so the src projection is a plain matmul term --
    no per-chunk indirect DMA, no transposes, no DVE add.
  - kv1 (+ negated feature-means from extra weight columns + negated ew
    logit) is materialized in PSUM by 3 accumulating matmuls, then one Act
    copy lands it in SBUF where Pool (gpsimd) can do the relu/normalize.
  - LN variance via tensor_tensor_reduce (sum x^2) + batched mean^2
    correction; rsqrt via exp(-0.5*ln(var+eps)) keeping the Act table fixed
    on {exp,ln,copy,relu,square}.
  - membership matrices built bf16 (4x DVE mode); logits/exp/weighted-v as
    in the baseline 3-phase scheme but with paired (2-chunk) DVE ops.
"""

import sys

for _p in ("/opt/trn_rl_repo",):
    if _p not in sys.path:
        sys.path.insert(0, _p)

import numpy as np

import concourse.bass as bass
import concourse.bacc as bacc
import concourse.tile as tile
from concourse import mybir
from concourse.bass_utils import run_bass_kernel_spmd
from concourse.masks import make_identity

N, E, D = 10000, 320000, 128
R, EF, NH = 64, 4, 16
DH = D // NH
NCORES = 8
NPC = N // NCORES
P = 128
NB = (NPC + P - 1) // P
NPAD = NB * P
EPS = 1e-5
F32 = mybir.dt.float32
I32 = mybir.dt.int32
I16 = mybir.dt.int16
BF16 = mybir.dt.bfloat16
AF = mybir.ActivationFunctionType
OP = mybir.AluOpType

GMAX = 896          # max num_idxs per dma_gather piece (HW cap < 1024)

LAST_RESULTS = None


def _prep(inputs):
    import ml_dtypes
    bf16 = ml_dtypes.bfloat16

    h = np.ascontiguousarray(inputs["h"], dtype=np.float32)
    r_feat = np.ascontiguousarray(inputs["r_feat"], dtype=np.float32)
    edge_feat = np.ascontiguousarray(inputs["edge_feat"], dtype=np.float32)
    ei = np.asarray(inputs["edge_index"])
    src = ei[0].astype(np.int64)
    dst = ei[1].astype(np.int64)

    perm = np.argsort(dst, kind="stable")
    sdst = dst[perm]
    counts = np.bincount(dst, minlength=N)
    cum = np.zeros(N + 1, dtype=np.int64)
    np.cumsum(counts, out=cum[1:])

    bstarts = np.empty((NCORES, NB), dtype=np.int64)
    bends = np.empty((NCORES, NB), dtype=np.int64)
    for c in range(NCORES):
        for b in range(NB):
            s = c * NPC + b * P
            e = min(s + P, (c + 1) * NPC)
            bstarts[c, b], bends[c, b] = s, e
    bcounts = cum[bends] - cum[bstarts]
    LT = int(((bcounts.max() + P - 1) // P) * P)
    EC = NB * LT

    f = lambda x: np.ascontiguousarray(np.asarray(x), dtype=np.float32)
    flags = {"ew_b": float(np.asarray(inputs["ew_b"]).reshape(-1)[0])}
    for nm in ("hk", "hv", "hq", "no"):
        g = f(inputs[nm + "_g"])
        be = f(inputs[nm + "_beta"])
        flags[nm + "_gb"] = not (np.all(g == 1.0) and np.all(be == 0.0))
    cb1 = np.concatenate([f(inputs["hk_b1"]), f(inputs["hv_b1"])])
    flags["cb1_nz"] = bool(np.any(cb1 != 0))
    flags["kb2_nz"] = bool(np.any(f(inputs["hk_b2"]) != 0))
    flags["vb2_nz"] = bool(np.any(f(inputs["hv_b2"]) != 0))
    other_b_zero = all(not np.any(f(inputs[k]) != 0) for k in
                       ("hq_b1", "hq_b2", "no_b1", "no_b2"))
    flags["fast"] = (not any(flags[nm + "_gb"] for nm in ("hk", "hv", "hq", "no"))
                    and not flags["cb1_nz"] and not flags["kb2_nz"]
                    and not flags["vb2_nz"] and other_b_zero)
    if not flags["fast"]:
        return None, LT, flags

    hk_w1, hv_w1 = f(inputs["hk_w1"]), f(inputs["hv_w1"])
    # input row blocks of W1: [edge_feat 0:EF | r_feat EF:EF+R | h_dst | h_src]
    Wk_dst, Wv_dst = hk_w1[EF + R:EF + R + D], hv_w1[EF + R:EF + R + D]
    Wk_src, Wv_src = hk_w1[EF + R + D:], hv_w1[EF + R + D:]
    # ref rows in refxT order: [r_feat (R) ; edge_feat (EF)]
    Wk_ref = np.concatenate([hk_w1[EF:EF + R], hk_w1[:EF]], 0)
    Wv_ref = np.concatenate([hv_w1[EF:EF + R], hv_w1[:EF]], 0)
    ew_w = f(inputs["ew_w"])[:, 0]  # [R]

    def kvx(Wk, Wv, extra=None):
        # [Wk | Wv | -mean(Wk) | -mean(Wv) | (extra)]
        cols = [Wk, Wv, -Wk.mean(1, keepdims=True), -Wv.mean(1, keepdims=True)]
        if extra is not None:
            cols.append(extra)
        return np.concatenate(cols, 1).astype(bf16)

    wdstx = kvx(Wk_dst, Wv_dst)                       # [128, 258]
    wsrcx = kvx(Wk_src, Wv_src)                       # [128, 258]
    ewneg = np.zeros((R + EF, 1), dtype=np.float32)
    ewneg[:R, 0] = -ew_w
    wrefx = kvx(Wk_ref, Wv_ref, ewneg)                # [68, 259]

    qscale = 1.0 / np.sqrt(DH)
    consts = {
        "wdstx": wdstx, "wsrcx": wsrcx, "wrefx": wrefx,
        "qw1b": f(inputs["hq_w1"]).astype(bf16),
        "qw2b": f(inputs["hq_w2"]).astype(bf16),
        "kw2b": f(inputs["hk_w2"]).astype(bf16),
        "vw2b": f(inputs["hv_w2"]).astype(bf16),
        "nw1ab": f(inputs["no_w1"])[:D].astype(bf16),
        "nw1bb": f(inputs["no_w1"])[D:].astype(bf16),
        "nw2b": f(inputs["no_w2"]).astype(bf16),
        "iotar": np.tile(np.arange(P, dtype=np.float32), (P, 1)).astype(bf16),
        "iotac": np.arange(P, dtype=np.float32)[:, None],
        "hb": h.astype(bf16),                         # [N, 128] gather table
    }
    NCH = LT // P

    in_maps = []
    for c in range(NCORES):
        dstrel = np.full(EC, -1000.0, dtype=np.float32)
        srci = np.zeros(EC, dtype=np.int16)
        refxT = np.zeros((R + EF, EC), dtype=bf16)
        for b in range(NB):
            lo, hi = cum[bstarts[c, b]], cum[bends[c, b]]
            L = hi - lo
            o = b * LT
            pidx = perm[lo:hi]
            dstrel[o:o + L] = (sdst[lo:hi] - bstarts[c, b]).astype(np.float32)
            srci[o:o + L] = src[pidx].astype(np.int16)
            refxT[:R, o:o + L] = r_feat[pidx].T
            refxT[R:, o:o + L] = edge_feat[pidx].T
        # wrap16 idx tables at partitions 16..31, one [128, LT//16] per bucket
        srcw = np.zeros((NB, 128, LT // 16), dtype=np.int16)
        for b in range(NB):
            srcw[b, 16:32, :] = srci[b * LT:(b + 1) * LT].reshape(LT // 16, 16).T
        hl = np.zeros((NPAD, D), dtype=np.float32)
        hl[:NPC] = h[c * NPC:(c + 1) * NPC]
        in_maps.append({
            "hl": hl,
            "dstrelb": dstrel.astype(bf16),
            "dstrelf": dstrel,
            "srcw": srcw,
            "refxT": refxT,
            **consts,
        })
    return in_maps, LT, flags


def _gpieces(LT):
    out, o = [], 0
    while o < LT:
        n = min(GMAX, LT - o)
        out.append((o, n))
        o += n
    return out


def _build_fast(LT, flags):
    NCH = LT // P
    NPAIR = NCH // 2
    assert NCH % 2 == 0
    nc = bacc.Bacc("TRN2", target_bir_lowering=False, detect_race_conditions=False)

    hl_d = nc.dram_tensor("hl", [NPAD, D], F32, kind="ExternalInput")
    hb_d = nc.dram_tensor("hb", [N, D], BF16, kind="ExternalInput")
    dstrelb_d = nc.dram_tensor("dstrelb", [NB * LT], BF16, kind="ExternalInput")
    dstrelf_d = nc.dram_tensor("dstrelf", [NB * LT], F32, kind="ExternalInput")
    srcw_d = nc.dram_tensor("srcw", [NB, 128, LT // 16], I16, kind="ExternalInput")
    refxT_d = nc.dram_tensor("refxT", [R + EF, NB * LT], BF16, kind="ExternalInput")
    cshapes = {
        "wdstx": ([D, 258], BF16), "wsrcx": ([D, 258], BF16),
        "wrefx": ([R + EF, 259], BF16),
        "qw1b": ([D, D], BF16), "qw2b": ([D, D], BF16),
        "kw2b": ([D, D], BF16), "vw2b": ([D, D], BF16),
        "nw1ab": ([D, D], BF16), "nw1bb": ([D, D], BF16), "nw2b": ([D, D], BF16),
        "iotar": ([P, P], BF16), "iotac": ([P, 1], F32),
    }
    cd = {k: nc.dram_tensor(k, sh, dt, kind="ExternalInput")
          for k, (sh, dt) in cshapes.items()}
    out_d = nc.dram_tensor("out", [NPC, D], F32, kind="ExternalOutput")
    import os as _os
    _DBG = bool(int(_os.environ.get("KDBG", "0")))
    if _DBG:
        dbg_kvq = nc.dram_tensor("dbg_kvq", [P, NCH * 387], F32, kind="ExternalOutput")
        dbg_lgall = nc.dram_tensor("dbg_lgall", [P, NCH * NH], F32, kind="ExternalOutput")
        dbg_v2all = nc.dram_tensor("dbg_v2all", [P, NCH * D], BF16, kind="ExternalOutput")
        dbg_stat = nc.dram_tensor("dbg_stat", [P, 2 * NCH], F32, kind="ExternalOutput")
        dbg_hsT = nc.dram_tensor("dbg_hsT", [P, LT], BF16, kind="ExternalOutput")
        dbg_agg = nc.dram_tensor("dbg_agg", [P, NH + D], F32, kind="ExternalOutput")
        dbg_exm = nc.dram_tensor("dbg_exm", [P, NH + D], BF16, kind="ExternalOutput")
        dbg_rsew = nc.dram_tensor("dbg_rsew", [P, NCH], F32, kind="ExternalOutput")
        dbg_exall = nc.dram_tensor("dbg_exall", [P, NCH * NH], BF16, kind="ExternalOutput")

    qscale = 1.0 / np.sqrt(DH)
    ew_b = flags["ew_b"]
    AW = 387
    AWS = AW          # A cols: k 0:128 | v 128:256 | -mk 256 | -mv 257 | -ew 258 | q 259:387

    with tile.TileContext(nc) as tc:
        with (
            tc.tile_pool(name="cpool", bufs=1) as cpool,
            tc.tile_pool(name="bpool", bufs=2) as bpool,
            tc.tile_pool(name="kpool", bufs=8) as kpool,
            tc.tile_pool(name="spool", bufs=4) as spool,
            tc.tile_pool(name="psum", bufs=1, space="PSUM") as ppool,
        ):
            cs = {}
            for k, (sh, dt) in cshapes.items():
                t = cpool.tile(sh, dt, tag="c_" + k)
                nc.sync.dma_start(out=t[:], in_=cd[k][:, :])
                cs[k] = t
            identb = cpool.tile([P, P], BF16, tag="identb")
            make_identity(nc, identb[:])
            ident = cpool.tile([P, P], F32, tag="ident")
            make_identity(nc, ident[:])
            epsc = cpool.tile([P, 1], F32, tag="epsc")
            nc.vector.memset(epsc[:], EPS)

            for b in range(NB):
                bs = min(P, NPC - b * P)
                # ---------------- bucket precompute ----------------
                hlt = bpool.tile([P, D], F32, tag="hlt")
                nc.sync.dma_start(out=hlt[:], in_=hl_d[b * P:(b + 1) * P, :])
                tpq = ppool.tile([P, AW], F32, tag="A", bufs=3, space="PSUM")
                nc.tensor.transpose(out=tpq[:, :P], in_=hlt[:], identity=ident[:])
                hTb = bpool.tile([P, P], BF16, tag="hTb")
                nc.scalar.activation(out=hTb[:], in_=tpq[:, :P], func=AF.Copy)

                Bd = bpool.tile([P, AW], BF16, tag="Bd")
                hw_ps = ppool.tile([P, AW], F32, tag="A", bufs=3, space="PSUM")
                hw_ps = hw_ps[:, :258]
                nc.tensor.matmul(out=hw_ps[:], lhsT=hTb[:], rhs=cs["wdstx"][:],
                                 start=True, stop=True)
                nc.scalar.activation(out=Bd[:, :258], in_=hw_ps[:], func=AF.Copy)
                nc.vector.memset(Bd[:, 258:259], -ew_b)

                # q MLP -> Bd[:, 259:387]  (baseline bn_stats approach)
                q1_ps = ppool.tile([P, AW], F32, tag="A", bufs=3, space="PSUM")
                q1_ps = q1_ps[:, :P]
                nc.tensor.matmul(out=q1_ps, lhsT=hTb[:], rhs=cs["qw1b"][:],
                                 start=True, stop=True)
                bsq = spool.tile([P, 6], F32, tag="bsq")
                nc.vector.bn_stats(out=bsq[:], in_=q1_ps)
                agq = spool.tile([P, 2], F32, tag="agq")
                nc.vector.bn_aggr(out=agq[:], in_=bsq[:])
                zq = bpool.tile([P, D], BF16, tag="zq")
                nc.vector.tensor_scalar(out=zq[:], in0=q1_ps,
                                        scalar1=agq[:, 0:1], scalar2=0.0,
                                        op0=OP.subtract, op1=OP.max)
                tpz = ppool.tile([P, 4 * P], BF16, tag="tpkv", bufs=2, space="PSUM")
                nc.tensor.transpose(out=tpz[:, :P], in_=zq[:], identity=identb[:])
                zqT = bpool.tile([P, P], BF16, tag="zqT")
                nc.scalar.activation(out=zqT[:], in_=tpz[:, :P], func=AF.Copy)
                q2_ps = ppool.tile([P, AW], F32, tag="A", bufs=3, space="PSUM")
                q2_ps = q2_ps[:, :P]
                nc.tensor.matmul(out=q2_ps, lhsT=zqT[:], rhs=cs["qw2b"][:],
                                 start=True, stop=True)
                # rs_q = exp(-0.5*ln(var+eps)); table stays in {exp,ln,...}
                sdq = spool.tile([P, 1], F32, tag="sdq")
                nc.scalar.activation(out=sdq[:], in_=agq[:, 1:2], func=AF.Sqrt,
                                     bias=epsc[:])
                rsq = spool.tile([P, 1], F32, tag="rsq")
                nc.vector.reciprocal(rsq[:], sdq[:])
                nc.vector.tensor_scalar(out=Bd[:, 259:], in0=q2_ps,
                                        scalar1=rsq[:], scalar2=qscale,
                                        op0=OP.mult, op1=OP.mult)

                # ---------------- bucket-wide loads / builds ----------------
                dstrow = bpool.tile([P, LT], BF16, tag="dstrow", bufs=1)
                nc.sync.dma_start(
                    out=dstrow[:],
                    in_=dstrelb_d[None, b * LT:(b + 1) * LT].to_broadcast([P, LT]))
                MnAll = bpool.tile([P, LT], BF16, tag="MnAll")
                nc.vector.tensor_scalar(out=MnAll[:], in0=dstrow[:],
                                        scalar1=cs["iotac"][:], scalar2=None,
                                        op0=OP.is_equal)
                dcolB = bpool.tile([P, NCH], F32, tag="dcolB")
                nc.sync.dma_start(
                    out=dcolB[:],
                    in_=dstrelf_d[b * LT:(b + 1) * LT].rearrange("(c p) -> p c", p=P))
                srcw = bpool.tile([128, LT // 16], I16, tag="srcw")
                nc.sync.dma_start(out=srcw[:], in_=srcw_d[b, :, :])
                hsg = bpool.tile([P, LT], BF16, tag="hsg", bufs=1)
                for gi, (o, n) in enumerate(_gpieces(LT)):
                    nc.gpsimd.dma_gather(
                        out_ap=hsg[:, o:o + n].rearrange("p (j d) -> p j d", d=D),
                        in_ap=hb_d[:, :], idxs_ap=srcw[:, o // 16:(o + n) // 16],
                        num_idxs=n, num_idxs_reg=n, elem_size=D, transpose=False)
                # transpose all chunks' src rows up front: hsTAll[d, e]
                hsTAll = bpool.tile([P, LT], BF16, tag="hsTAll")
                for g in range(0, NCH, 4):
                    ng = min(4, NCH - g)
                    tpg = ppool.tile([P, 4 * P], BF16, tag="tpkv", bufs=2,
                                     space="PSUM")
                    for t in range(ng):
                        nc.tensor.transpose(
                            out=tpg[:, t * P:(t + 1) * P],
                            in_=hsg[:, (g + t) * P:(g + t + 1) * P],
                            identity=identb[:])
                    nc.vector.tensor_copy(
                        out=hsTAll[:, g * P:(g + ng) * P],
                        in_=tpg[:, :ng * P])
                refT = bpool.tile([R + EF, LT], BF16, tag="refT")
                nc.sync.dma_start(out=refT[:], in_=refxT_d[:, b * LT:(b + 1) * LT])

                # bucket stashes
                kvq = bpool.tile([P, NCH * AWS], BF16, tag="kvq", bufs=2)
                MTall = bpool.tile([P, NCH * P], BF16, tag="MTall")
                v2all = bpool.tile([P, NCH * D], BF16, tag="v2all")
                lgall = bpool.tile([P, NCH * NH], F32, tag="lgall")
                statkv = bpool.tile([P, 2 * NCH], F32, tag="statkv")
                sqscr = bpool.tile([P, D], BF16, tag="sqscr")

                # ---------------- phase A ----------------
                import os as _os
                _STOP = int(_os.environ.get("KSTOP", "99"))
                if _STOP < 1:
                    outt0 = bpool.tile([P, D], F32, tag="outt")
                    nc.vector.tensor_tensor(out=outt0[:], in0=hlt[:], in1=hlt[:],
                                            op=OP.add)
                    nc.sync.dma_start(out=out_d[b * P:b * P + bs, :],
                                      in_=outt0[:bs, :])
                    continue
                for ci in range(NCH):
                    e0 = ci * P
                    A = ppool.tile([P, AW], F32, tag="A", bufs=3, space="PSUM")
                    nc.tensor.matmul(out=A[:], lhsT=MnAll[:, e0:e0 + P],
                                     rhs=Bd[:], start=True, stop=False)
                    nc.tensor.matmul(out=A[:, :258], lhsT=hsTAll[:, e0:e0 + P],
                                     rhs=cs["wsrcx"][:], start=False, stop=False)
                    nc.tensor.matmul(out=A[:, :259], lhsT=refT[:, e0:e0 + P],
                                     rhs=cs["wrefx"][:], start=False, stop=True)
                    if ci % 4 == 0:
                        g4 = min(4, NCH - ci)
                        nc.vector.tensor_tensor(
                            out=MTall[:, e0:e0 + g4 * P].rearrange(
                                "p (c j) -> p c j", c=g4),
                            in0=cs["iotar"][:][:, None, :].to_broadcast(
                                [P, g4, P]),
                            in1=dcolB[:, ci:ci + g4][:, :, None].to_broadcast(
                                [P, g4, P]),
                            op=OP.is_equal)
                    # land kv1 (k|v|negmk|negmv|negew|q) in SBUF
                    kv = kvq[:, ci * AWS:(ci + 1) * AWS]
                    nc.scalar.activation(out=kv, in_=A[:], func=AF.Copy)
                    kcol = kvq[:, ci * AWS + 256:ci * AWS + 257]
                    vcol = kvq[:, ci * AWS + 257:ci * AWS + 258]
                    # centered sum of squares per half: Square(x + negmu)
                    nc.scalar.activation(out=sqscr[:], in_=kv[:, :D],
                                         func=AF.Square, bias=kcol,
                                         accum_out=statkv[:, 2 * ci:2 * ci + 1])
                    nc.scalar.activation(out=sqscr[:], in_=kv[:, D:256],
                                         func=AF.Square, bias=vcol,
                                         accum_out=statkv[:, 2 * ci + 1:2 * ci + 2])
                    # relu(x - mu): TT add with broadcast negmu, then relu
                    zkv = kpool.tile([P, 2 * D], BF16, tag="zkv")
                    nc.vector.tensor_tensor(
                        out=zkv[:].rearrange("p (t d) -> p t d", t=2),
                        in0=kv[:, :256].rearrange("p (t d) -> p t d", t=2),
                        in1=kv[:, 256:258].rearrange(
                            "p (t o) -> p t o", t=2).to_broadcast([P, 2, D]),
                        op=OP.add)
                    nc.scalar.activation(out=zkv[:], in_=zkv[:], func=AF.Relu)
                    j = ci % 2
                    if j == 0:
                        tpkv = ppool.tile([P, 4 * P], BF16, tag="tpkv", bufs=2,
                                          space="PSUM")
                        kv2 = ppool.tile([P, 4 * D], F32, tag="kv2", bufs=2,
                                         space="PSUM")
                        zkvT = kpool.tile([P, 4 * P], BF16, tag="zkvT")
                    nc.tensor.transpose(out=tpkv[:, 2 * j * P:(2 * j + 1) * P],
                                        in_=zkv[:, :D], identity=identb[:])
                    nc.tensor.transpose(out=tpkv[:, (2 * j + 1) * P:(2 * j + 2) * P],
                                        in_=zkv[:, D:], identity=identb[:])
                    if j == 1 or ci == NCH - 1:
                        w = 2 * P * (j + 1)
                        nc.vector.tensor_copy(out=zkvT[:, :w], in_=tpkv[:, :w])
                        for t in range(j + 1):
                            nc.tensor.matmul(
                                out=kv2[:, 2 * t * D:(2 * t + 1) * D],
                                lhsT=zkvT[:, 2 * t * P:(2 * t + 1) * P],
                                rhs=cs["kw2b"][:], start=True, stop=True)
                            nc.tensor.matmul(
                                out=kv2[:, (2 * t + 1) * D:(2 * t + 2) * D],
                                lhsT=zkvT[:, (2 * t + 1) * P:(2 * t + 2) * P],
                                rhs=cs["vw2b"][:], start=True, stop=True)
                        nch = j + 1
                        c0 = ci - j
                        # logits: q (SBUF f32) * k2 (PSUM) -> reduce per head
                        lgscr = kpool.tile([P, 2 * D], F32, tag="lgscr")
                        nc.vector.tensor_tensor(
                            out=lgscr[:, :nch * D].rearrange(
                                "p (c d) -> p c d", c=nch),
                            in0=kvq[:, c0 * AWS:(c0 + nch) * AWS].rearrange(
                                "p (c w) -> p c w", c=nch)[:, :, 259:387],
                            in1=kv2[:, :nch * 2 * D].rearrange(
                                "p (c d) -> p c d", c=nch)[:, :, :D],
                            op=OP.mult)
                        nc.vector.tensor_reduce(
                            out=lgall[:, c0 * NH:(c0 + nch) * NH],
                            in_=lgscr[:, :nch * D].rearrange(
                                "p (c h d) -> p c h d", c=nch, d=DH),
                            axis=mybir.AxisListType.X, op=OP.add)
                        nc.scalar.activation(
                            out=v2all[:, c0 * D:(c0 + nch) * D].rearrange(
                                "p (c d) -> p c d", c=nch),
                            in_=kv2[:, :nch * 2 * D].rearrange(
                                "p (c d) -> p c d", c=nch)[:, :, D:],
                            func=AF.Copy)

                # ---------------- phase B (batched scalars) ----------------
                if _STOP < 2:
                    outt0 = bpool.tile([P, D], F32, tag="outt")
                    nc.vector.tensor_tensor(out=outt0[:], in0=hlt[:], in1=hlt[:],
                                            op=OP.add)
                    nc.sync.dma_start(out=out_d[b * P:b * P + bs, :],
                                      in_=outt0[:bs, :])
                    continue
                varkv = bpool.tile([P, 2 * NCH], F32, tag="varkv")
                nc.vector.tensor_scalar(out=varkv[:], in0=statkv[:],
                                        scalar1=1.0 / D, scalar2=None, op0=OP.mult)
                sdall = bpool.tile([P, 2 * NCH], F32, tag="sdall")
                nc.scalar.activation(out=sdall[:], in_=varkv[:], func=AF.Sqrt,
                                     bias=epsc[:])
                rsall = bpool.tile([P, 2 * NCH], F32, tag="rsall")
                nc.vector.reciprocal(rsall[:], sdall[:])
                # sigmoid(ew) from negated logit: 1/(1+exp(negew))
                ewx = bpool.tile([P, NCH], F32, tag="ewx")
                nc.scalar.activation(
                    out=ewx[:][:, :, None],
                    in_=kvq[:].rearrange("p (c w) -> p c w", w=AWS)[:, :, 258:259],
                    func=AF.Exp)
                nc.vector.tensor_scalar(out=ewx[:], in0=ewx[:], scalar1=1.0,
                                        scalar2=None, op0=OP.add)
                nc.vector.reciprocal(ewx[:], ewx[:])
                rsew = bpool.tile([P, NCH], F32, tag="rsew")
                nc.vector.tensor_tensor(out=rsew[:], in0=ewx[:],
                                        in1=rsall[:, 1::2], op=OP.mult)
                nc.vector.tensor_tensor(
                    out=lgall[:].rearrange("p (c h) -> p c h", h=NH),
                    in0=lgall[:].rearrange("p (c h) -> p c h", h=NH),
                    in1=rsall[:, 0::2][:, :, None].to_broadcast([P, NCH, NH]),
                    op=OP.mult)
                exall = bpool.tile([P, NCH * NH], BF16, tag="exall")
                half = (NCH // 2) * NH
                nc.scalar.activation(out=exall[:, :half], in_=lgall[:, :half],
                                     func=AF.Exp)
                nc.scalar.activation(out=exall[:, half:], in_=lgall[:, half:],
                                     func=AF.Exp)
                wgtvA = bpool.tile([P, NCH * NH], BF16, tag="wgtvA")
                nc.vector.tensor_tensor(
                    out=wgtvA[:].rearrange("p (c h) -> p c h", h=NH),
                    in0=exall[:].rearrange("p (c h) -> p c h", h=NH),
                    in1=rsew[:][:, :, None].to_broadcast([P, NCH, NH]),
                    op=OP.mult)

                # ---------------- phase C ----------------
                if _STOP < 3:
                    outt0 = bpool.tile([P, D], F32, tag="outt")
                    nc.vector.tensor_tensor(out=outt0[:], in0=hlt[:], in1=hlt[:],
                                            op=OP.add)
                    nc.sync.dma_start(out=out_d[b * P:b * P + bs, :],
                                      in_=outt0[:bs, :])
                    continue
                agg = ppool.tile([P, NH + D], F32, tag="agg", bufs=1, space="PSUM")
                for pi in range(NCH // 2):
                    exm = kpool.tile([P, 2 * (NH + D)], BF16, tag="exm")
                    c0 = 2 * pi
                    nc.vector.tensor_copy(
                        out=exm[:].rearrange("p (c f) -> p c f", c=2)[:, :, :NH],
                        in_=exall[:, c0 * NH:(c0 + 2) * NH].rearrange(
                            "p (c h) -> p c h", c=2))
                    nc.vector.tensor_tensor(
                        out=exm[:].rearrange(
                            "p (c f) -> p c f", c=2)[:, :, NH:].rearrange(
                            "p c (h d) -> p c h d", d=DH),
                        in0=v2all[:, c0 * D:(c0 + 2) * D].rearrange(
                            "p (c h d) -> p c h d", c=2, d=DH),
                        in1=wgtvA[:, c0 * NH:(c0 + 2) * NH].rearrange(
                            "p (c h) -> p c h", c=2)[:, :, :, None].to_broadcast(
                            [P, 2, NH, DH]),
                        op=OP.mult)
                    if _DBG and b == 0 and pi == 0:
                        nc.sync.dma_start(out=dbg_exm[:, :], in_=exm[:, :NH + D])
                        nc.sync.dma_start(out=dbg_rsew[:, :], in_=rsew[:])
                        nc.sync.dma_start(out=dbg_exall[:, :], in_=exall[:])
                    for t in range(2):
                        ci = c0 + t
                        nc.tensor.matmul(
                            out=agg[:], lhsT=MTall[:, ci * P:(ci + 1) * P],
                            rhs=exm[:, t * (NH + D):(t + 1) * (NH + D)],
                            start=(ci == 0), stop=(ci == NCH - 1),
                            skip_group_check=True)

                # ---------------- bucket epilogue ----------------
                den = bpool.tile([P, NH], F32, tag="den")
                nc.vector.tensor_scalar_max(den[:], agg[:, :NH], 1e-30)
                rd = bpool.tile([P, NH], F32, tag="rd")
                nc.vector.reciprocal(rd[:], den[:])
                attn = bpool.tile([P, D], F32, tag="attn")
                nc.vector.tensor_tensor(
                    out=attn[:].rearrange("p (h d) -> p h d", d=DH),
                    in0=agg[:, NH:].rearrange("p (h d) -> p h d", d=DH),
                    in1=rd[:][:, :, None].to_broadcast([P, NH, DH]),
                    op=OP.mult)
                tpa = ppool.tile([P, AW], F32, tag="A", bufs=3, space="PSUM")
                nc.tensor.transpose(out=tpa[:, :P], in_=attn[:], identity=ident[:])
                attnT = bpool.tile([P, P], BF16, tag="attnT")
                nc.scalar.activation(out=attnT[:], in_=tpa[:, :P], func=AF.Copy)

                f1_ps = ppool.tile([P, AW], F32, tag="A", bufs=3, space="PSUM")
                f1_ps = f1_ps[:, :P]
                nc.tensor.matmul(out=f1_ps, lhsT=attnT[:], rhs=cs["nw1ab"][:],
                                 start=True, stop=False)
                nc.tensor.matmul(out=f1_ps, lhsT=hTb[:], rhs=cs["nw1bb"][:],
                                 start=False, stop=True)
                bsf = spool.tile([P, 6], F32, tag="bsf")
                nc.vector.bn_stats(out=bsf[:], in_=f1_ps)
                agf = spool.tile([P, 2], F32, tag="agf")
                nc.vector.bn_aggr(out=agf[:], in_=bsf[:])
                zf = bpool.tile([P, D], BF16, tag="zf")
                nc.vector.tensor_scalar(out=zf[:], in0=f1_ps,
                                        scalar1=agf[:, 0:1], scalar2=0.0,
                                        op0=OP.subtract, op1=OP.max)
                tpf2 = ppool.tile([P, 4 * P], BF16, tag="tpkv", bufs=2, space="PSUM")
                nc.tensor.transpose(out=tpf2[:, :P], in_=zf[:], identity=identb[:])
                fzT = bpool.tile([P, P], BF16, tag="fzT")
                nc.scalar.activation(out=fzT[:], in_=tpf2[:, :P], func=AF.Copy)
                f2_ps = ppool.tile([P, AW], F32, tag="A", bufs=3, space="PSUM")
                f2_ps = f2_ps[:, :P]
                nc.tensor.matmul(out=f2_ps, lhsT=fzT[:], rhs=cs["nw2b"][:],
                                 start=True, stop=True)
                sdf = spool.tile([P, 1], F32, tag="sdf")
                nc.scalar.activation(out=sdf[:], in_=agf[:, 1:2], func=AF.Sqrt,
                                     bias=epsc[:])
                rsf = spool.tile([P, 1], F32, tag="rsf")
                nc.vector.reciprocal(rsf[:], sdf[:])
                t1 = bpool.tile([P, D], F32, tag="t1")
                nc.vector.tensor_scalar(out=t1[:], in0=f2_ps,
                                        scalar1=rsf[:], scalar2=None, op0=OP.mult)
                outt = bpool.tile([P, D], F32, tag="outt")
                nc.vector.tensor_tensor(out=outt[:], in0=t1[:], in1=hlt[:],
                                        op=OP.add)
                nc.sync.dma_start(out=out_d[b * P:b * P + bs, :], in_=outt[:bs, :])
    nc.compile()
    return nc


def kernel(**inputs):
    global LAST_RESULTS
    in_maps, LT, flags = _prep(inputs)
    if in_maps is None:
        in_maps_g, LTg, flags_g = _prep_general(inputs)
        nc = _build_general(LTg, flags_g)
        import os
        trace = bool(int(os.environ.get("KBENCH_TRACE", "0")))
        res = run_bass_kernel_spmd(nc, in_maps_g, core_ids=list(range(NCORES)),
                                   trace=trace)
        LAST_RESULTS = res
        outs = res.results
        return np.concatenate([outs[c]["out"] for c in range(NCORES)],
                              axis=0).astype(np.float32)
    nc = _build_fast(LT, flags)
    import os
    trace = bool(int(os.environ.get("KBENCH_TRACE", "0")))
    res = run_bass_kernel_spmd(nc, in_maps, core_ids=list(range(NCORES)),
                               trace=trace)
    LAST_RESULTS = res
    outs = res.results
    full = np.concatenate([outs[c]["out"] for c in range(NCORES)], axis=0)
    return full.astype(np.float32)


def _prep_general(inputs):
    h = np.ascontiguousarray(inputs["h"], dtype=np.float32)
    r_feat = np.ascontiguousarray(inputs["r_feat"], dtype=np.float32)
    edge_feat = np.ascontiguousarray(inputs["edge_feat"], dtype=np.float32)
    ei = np.asarray(inputs["edge_index"])
    src = ei[0].astype(np.int64)
    dst = ei[1].astype(np.int64)

    perm = np.argsort(dst, kind="stable")
    sdst = dst[perm]
    counts = np.bincount(dst, minlength=N)
    cum = np.zeros(N + 1, dtype=np.int64)
    np.cumsum(counts, out=cum[1:])

    # bucket (core c, bucket b) covers global nodes [c*NPC + b*P, min(..+P, (c+1)*NPC))
    bstarts = np.empty((NCORES, NB), dtype=np.int64)
    bends = np.empty((NCORES, NB), dtype=np.int64)
    for c in range(NCORES):
        for b in range(NB):
            s = c * NPC + b * P
            e = min(s + P, (c + 1) * NPC)
            bstarts[c, b], bends[c, b] = s, e
    bcounts = cum[bends] - cum[bstarts]
    LT = int(((bcounts.max() + P - 1) // P) * P)
    EC = NB * LT

    in_maps = []
    for c in range(NCORES):
        dstrel = np.full(EC, -1000.0, dtype=np.float32)
        srci = np.zeros(EC, dtype=np.int32)
        refx = np.zeros((EC, R + EF), dtype=np.float32)
        for b in range(NB):
            lo, hi = cum[bstarts[c, b]], cum[bends[c, b]]
            L = hi - lo
            o = b * LT
            pidx = perm[lo:hi]
            dstrel[o:o + L] = (sdst[lo:hi] - bstarts[c, b]).astype(np.float32)
            srci[o:o + L] = src[pidx].astype(np.int32)
            refx[o:o + L, :R] = r_feat[pidx]
            refx[o:o + L, R:] = edge_feat[pidx]
        hl = np.zeros((NPAD, D), dtype=np.float32)
        hl[:NPC] = h[c * NPC:(c + 1) * NPC]
        in_maps.append({
            "h": h, "hl": hl, "dstrel": dstrel, "srci": srci, "refx": refx,
        })

    f = lambda x: np.ascontiguousarray(np.asarray(x), dtype=np.float32)
    hk_w1, hv_w1 = f(inputs["hk_w1"]), f(inputs["hv_w1"])
    wdst = np.concatenate([hk_w1[EF + R:EF + R + D], hv_w1[EF + R:EF + R + D]], 1)
    wsrc = np.concatenate([hk_w1[EF + R + D:], hv_w1[EF + R + D:]], 1)
    wref = np.zeros((R + EF, 2 * D + 1), dtype=np.float32)
    wref[:R, :D] = hk_w1[EF:EF + R]
    wref[:R, D:2 * D] = hv_w1[EF:EF + R]
    wref[R:, :D] = hk_w1[:EF]
    wref[R:, D:2 * D] = hv_w1[:EF]
    wref[:R, 2 * D] = f(inputs["ew_w"])[:, 0]
    cb1 = np.concatenate([f(inputs["hk_b1"]), f(inputs["hv_b1"])])[None, :]  # [1,256]
    ew_b = float(np.asarray(inputs["ew_b"]).reshape(-1)[0])

    consts = {
        "wdst": wdst, "wsrc": wsrc, "wref": wref, "cb1": cb1,
        "qw1": f(inputs["hq_w1"]), "qb1": f(inputs["hq_b1"])[None, :],
        "qw2": f(inputs["hq_w2"]), "qb2": f(inputs["hq_b2"])[None, :],
        "kw2": f(inputs["hk_w2"]), "kb2": f(inputs["hk_b2"])[None, :],
        "vw2": f(inputs["hv_w2"]), "vb2": f(inputs["hv_b2"])[None, :],
        "nw1a": f(inputs["no_w1"])[:D], "nw1b": f(inputs["no_w1"])[D:],
        "nb1": f(inputs["no_b1"])[None, :],
        "nw2": f(inputs["no_w2"]), "nb2": f(inputs["no_b2"])[None, :],
        "iotar": np.tile(np.arange(P, dtype=np.float32), (P, 1)),
    }
    gb = {}
    flags = {"ew_b": ew_b}
    for nm in ("hk", "hv", "hq", "no"):
        g = f(inputs[nm + "_g"])
        be = f(inputs[nm + "_beta"])
        trivial = bool(np.all(g == 1.0) and np.all(be == 0.0))
        flags[nm + "_gb"] = not trivial
        if not trivial:
            gb[nm + "_grep"] = np.tile(g[None, :], (P, 1))
            gb[nm + "_brep"] = np.tile(be[None, :], (P, 1))
    flags["cb1_nz"] = bool(np.any(cb1 != 0))
    flags["kb2_nz"] = bool(np.any(consts["kb2"] != 0))
    flags["vb2_nz"] = bool(np.any(consts["vb2"] != 0))
    other_b_zero = all(not np.any(consts[k] != 0) for k in
                       ("qb1", "qb2", "nb1", "nb2"))
    flags["fast"] = (not any(flags[nm + "_gb"] for nm in ("hk", "hv", "hq", "no"))
                     and not flags["cb1_nz"] and not flags["kb2_nz"]
                     and not flags["vb2_nz"] and other_b_zero)
    consts.update(gb)
    if not flags["fast"]:
        for m in in_maps:
            m.update(consts)
        return in_maps, LT, flags

    # ---- fast path arrays (bf16 matmul operands, pre-transposed/pre-projected) ----
    import ml_dtypes
    bf16 = ml_dtypes.bfloat16
    NCH = LT // P
    hsw = (h @ wsrc).astype(bf16)                       # [N, 256] src projection table
    fc = {
        "hsw": hsw,
        "wdstb": wdst.astype(bf16),
        "wrefb": wref.astype(bf16),
        "qw1b": consts["qw1"].astype(bf16), "qw2b": consts["qw2"].astype(bf16),
        "kw2b": consts["kw2"].astype(bf16), "vw2b": consts["vw2"].astype(bf16),
        "nw1ab": consts["nw1a"].astype(bf16), "nw1bb": consts["nw1b"].astype(bf16),
        "nw2b": consts["nw2"].astype(bf16),
        "iotar": consts["iotar"],
        "iotac": np.arange(P, dtype=np.float32)[:, None],
    }
    fast_maps = []
    for c, m in enumerate(in_maps):
        refxT = np.zeros((NB * NCH, R + EF, P), dtype=bf16)
        rx = m["refx"].reshape(NB * NCH, P, R + EF)
        refxT[:] = rx.transpose(0, 2, 1).astype(bf16)
        fast_maps.append({
            "hl": m["hl"],
            "dstrelb": m["dstrel"].astype(bf16),
            "dstrelf": m["dstrel"],
            "srci": m["srci"],
            "refxT": refxT,
            **fc,
        })
    return fast_maps, LT, flags


def _build_general(LT, flags):
    NCH = LT // P  # chunks per bucket
    nc = bacc.Bacc("TRN2", target_bir_lowering=False, detect_race_conditions=False)

    h_d = nc.dram_tensor("h", [N, D], F32, kind="ExternalInput")
    hl_d = nc.dram_tensor("hl", [NPAD, D], F32, kind="ExternalInput")
    dstrel_d = nc.dram_tensor("dstrel", [NB * LT], F32, kind="ExternalInput")
    srci_d = nc.dram_tensor("srci", [NB * LT], I32, kind="ExternalInput")
    refx_d = nc.dram_tensor("refx", [NB * LT, R + EF], F32, kind="ExternalInput")
    cd = {}
    cshapes = {
        "wdst": [D, 2 * D], "wsrc": [D, 2 * D], "wref": [R + EF, 2 * D + 1],
        "cb1": [1, 2 * D], "qw1": [D, D], "qb1": [1, D], "qw2": [D, D],
        "qb2": [1, D], "kw2": [D, D], "kb2": [1, D], "vw2": [D, D],
        "vb2": [1, D], "nw1a": [D, D], "nw1b": [D, D], "nb1": [1, D], "nw2": [D, D],
        "nb2": [1, D], "iotar": [P, P],
    }
    for nm in ("hk", "hv", "hq", "no"):
        if flags[nm + "_gb"]:
            cshapes[nm + "_grep"] = [P, D]
            cshapes[nm + "_brep"] = [P, D]
    for k, s in cshapes.items():
        cd[k] = nc.dram_tensor(k, s, F32, kind="ExternalInput")
    out_d = nc.dram_tensor("out", [NPC, D], F32, kind="ExternalOutput")

    qscale = 1.0 / np.sqrt(DH)

    with tile.TileContext(nc) as tc:
        with (
            tc.tile_pool(name="cpool", bufs=1) as cpool,
            tc.tile_pool(name="bpool", bufs=2) as bpool,
            tc.tile_pool(name="kpool", bufs=3) as kpool,
            tc.tile_pool(name="spool", bufs=4) as spool,
            tc.tile_pool(name="psum", bufs=1, space="PSUM") as ppool,
        ):
            # ---- constants resident in SBUF ----
            cs = {}
            for k, s in cshapes.items():
                t = cpool.tile(s, F32, tag="c_" + k)
                nc.sync.dma_start(out=t[:], in_=cd[k][:, :])
                cs[k] = t
            ident = cpool.tile([P, P], F32, tag="ident")
            make_identity(nc, ident[:])
            ones1 = cpool.tile([1, P], F32, tag="ones1")
            nc.vector.memset(ones1[:], 1.0)
            epsc = cpool.tile([P, 1], F32, tag="epsc")
            nc.vector.memset(epsc[:], EPS)

            def ln_relu(x_psum, out_sb, pref):
                """out_sb = relu(layernorm(x_psum) * g + beta), per-partition stats."""
                scr = spool.tile([P, P], F32, tag="scr")
                s1 = spool.tile([P, 1], F32, tag="s1")
                nc.scalar.activation(out=scr[:], in_=x_psum, func=AF.Copy,
                                     accum_out=s1[:])
                scr2 = spool.tile([P, P], F32, tag="scr2")
                s2 = spool.tile([P, 1], F32, tag="s2")
                nc.scalar.activation(out=scr2[:], in_=x_psum, func=AF.Square,
                                     accum_out=s2[:])
                mu = spool.tile([P, 1], F32, tag="mu")
                nc.vector.tensor_scalar_mul(mu[:], s1[:], 1.0 / D)
                var = spool.tile([P, 1], F32, tag="var")
                nc.vector.tensor_scalar(out=var[:], in0=s2[:], scalar1=1.0 / D,
                                        scalar2=None, op0=OP.mult)
                mu2 = spool.tile([P, 1], F32, tag="mu2")
                nc.vector.tensor_tensor(out=mu2[:], in0=mu[:], in1=mu[:], op=OP.mult)
                nc.vector.tensor_tensor(out=var[:], in0=var[:], in1=mu2[:],
                                        op=OP.subtract)
                sd = spool.tile([P, 1], F32, tag="sd")
                nc.scalar.activation(out=sd[:], in_=var[:], func=AF.Sqrt, bias=epsc[:])
                rs = spool.tile([P, 1], F32, tag="rs")
                nc.vector.reciprocal(rs[:], sd[:])
                nc.vector.tensor_scalar(out=out_sb, in0=x_psum, scalar1=mu[:],
                                        scalar2=rs[:], op0=OP.subtract, op1=OP.mult)
                if flags[pref + "_gb"]:
                    nc.vector.tensor_tensor(out=out_sb, in0=out_sb,
                                            in1=cs[pref + "_grep"][:], op=OP.mult)
                    nc.vector.tensor_tensor(out=out_sb, in0=out_sb,
                                            in1=cs[pref + "_brep"][:], op=OP.add)
                nc.vector.tensor_scalar_max(out_sb, out_sb, 0.0)

            def transpose_to_sb(src_sb, out_sb, np_, nf):
                """PE-transpose src_sb[:np_, :nf] -> out_sb[:nf, :np_] via PSUM."""
                tp = ppool.tile([P, P], F32, tag="tp", space="PSUM")
                nc.tensor.transpose(out=tp[:nf, :np_], in_=src_sb, identity=ident[:])
                nc.scalar.activation(out=out_sb, in_=tp[:nf, :np_], func=AF.Copy)

            for b in range(NB):
                bs = min(P, NPC - b * P)
                # ---------- bucket precompute ----------
                hlt = bpool.tile([P, D], F32, tag="hlt")
                nc.sync.dma_start(out=hlt[:], in_=hl_d[b * P:(b + 1) * P, :])
                hT = bpool.tile([P, P], F32, tag="hT")
                transpose_to_sb(hlt[:], hT[:], P, P)

                Bd = bpool.tile([P, 2 * D + 1 + D], F32, tag="Bd")  # [128, 385]

                # hW_dst = h_tile @ W1_dst (+ b1)  -> Bd[:, 0:256]
                hw_ps = ppool.tile([P, 2 * D], F32, tag="A", space="PSUM")
                nc.tensor.matmul(out=hw_ps[:], lhsT=hT[:], rhs=cs["wdst"][:],
                                 start=True, stop=not flags["cb1_nz"])
                if flags["cb1_nz"]:
                    nc.tensor.matmul(out=hw_ps[:], lhsT=ones1[:], rhs=cs["cb1"][:],
                                     start=False, stop=True)
                nc.scalar.activation(out=Bd[:, :2 * D], in_=hw_ps[:], func=AF.Copy)
                nc.vector.memset(Bd[:, 2 * D:2 * D + 1], flags["ew_b"])

                # q = MLP_q(h_tile) * qscale -> Bd[:, 257:385]
                q1_ps = ppool.tile([P, 2 * D], F32, tag="A", space="PSUM")
                nc.tensor.matmul(out=q1_ps[:, :D], lhsT=hT[:], rhs=cs["qw1"][:],
                                 start=True, stop=False)
                nc.tensor.matmul(out=q1_ps[:, :D], lhsT=ones1[:], rhs=cs["qb1"][:],
                                 start=False, stop=True)
                qz = bpool.tile([P, D], F32, tag="qz")
                ln_relu(q1_ps[:, :D], qz[:], "hq")
                qzT = bpool.tile([P, P], F32, tag="qzT")
                transpose_to_sb(qz[:], qzT[:], P, P)
                q2_ps = ppool.tile([P, 2 * D], F32, tag="A", space="PSUM")
                nc.tensor.matmul(out=q2_ps[:, :D], lhsT=qzT[:], rhs=cs["qw2"][:],
                                 start=True, stop=False)
                nc.tensor.matmul(out=q2_ps[:, :D], lhsT=ones1[:], rhs=cs["qb2"][:],
                                 start=False, stop=True)
                nc.scalar.activation(out=Bd[:, 2 * D + 1:], in_=q2_ps[:, :D],
                                     func=AF.Copy, scale=qscale)

                agg = ppool.tile([P, NH + D], F32, tag="agg", space="PSUM")

                # ---------- edge chunks ----------
                for ci in range(NCH):
                    e0 = b * LT + ci * P
                    dcol = kpool.tile([P, 1], F32, tag="dcol")
                    nc.sync.dma_start(out=dcol[:], in_=dstrelf_d[e0:e0 + P, None])
                    scol = kpool.tile([P, 1], I32, tag="scol")
                    nc.sync.dma_start(out=scol[:], in_=srci_d[e0:e0 + P, None])
                    refx = kpool.tile([P, R + EF], F32, tag="refx")
                    nc.sync.dma_start(out=refx[:], in_=refx_d[e0:e0 + P, :])
                    hsrc = kpool.tile([P, D], F32, tag="hsrc")
                    nc.gpsimd.indirect_dma_start(
                        out=hsrc[:], out_offset=None, in_=h_d[:, :],
                        in_offset=bass.IndirectOffsetOnAxis(ap=scol[:, :1], axis=0))

                    MT = kpool.tile([P, P], F32, tag="MT")
                    nc.vector.tensor_scalar(out=MT[:], in0=cs["iotar"][:],
                                            scalar1=dcol[:], scalar2=None,
                                            op0=OP.is_equal)
                    Mn = kpool.tile([P, P], F32, tag="Mn")
                    transpose_to_sb(MT[:], Mn[:], P, P)
                    hsT = kpool.tile([P, P], F32, tag="hsT")
                    transpose_to_sb(hsrc[:], hsT[:], P, P)
                    refT = kpool.tile([R + EF, P], F32, tag="refT")
                    transpose_to_sb(refx[:], refT[:], P, R + EF)

                    A = ppool.tile([P, 2 * D + 1 + D], F32, tag="A", space="PSUM")
                    nc.tensor.matmul(out=A[:], lhsT=Mn[:], rhs=Bd[:],
                                     start=True, stop=False)
                    nc.tensor.matmul(out=A[:, :2 * D], lhsT=hsT[:], rhs=cs["wsrc"][:],
                                     start=False, stop=False)
                    nc.tensor.matmul(out=A[:, :2 * D + 1], lhsT=refT[:],
                                     rhs=cs["wref"][:], start=False, stop=True)

                    zk = kpool.tile([P, D], F32, tag="zk")
                    ln_relu(A[:, :D], zk[:], "hk")
                    zv = kpool.tile([P, D], F32, tag="zv")
                    ln_relu(A[:, D:2 * D], zv[:], "hv")
                    zkT = kpool.tile([P, P], F32, tag="zkT")
                    transpose_to_sb(zk[:], zkT[:], P, P)
                    zvT = kpool.tile([P, P], F32, tag="zvT")
                    transpose_to_sb(zv[:], zvT[:], P, P)

                    k2 = ppool.tile([P, D], F32, tag="k2", space="PSUM")
                    nc.tensor.matmul(out=k2[:], lhsT=zkT[:], rhs=cs["kw2"][:],
                                     start=True, stop=not flags["kb2_nz"])
                    if flags["kb2_nz"]:
                        nc.tensor.matmul(out=k2[:], lhsT=ones1[:], rhs=cs["kb2"][:],
                                         start=False, stop=True)
                    v2 = ppool.tile([P, D], F32, tag="v2", space="PSUM")
                    nc.tensor.matmul(out=v2[:], lhsT=zvT[:], rhs=cs["vw2"][:],
                                     start=True, stop=not flags["vb2_nz"])
                    if flags["vb2_nz"]:
                        nc.tensor.matmul(out=v2[:], lhsT=ones1[:], rhs=cs["vb2"][:],
                                         start=False, stop=True)

                    ew = kpool.tile([P, 1], F32, tag="ew")
                    nc.scalar.activation(out=ew[:], in_=A[:, 2 * D:2 * D + 1],
                                         func=AF.Sigmoid)
                    k2s = kpool.tile([P, D], F32, tag="k2s")
                    nc.scalar.activation(out=k2s[:], in_=k2[:], func=AF.Copy)
                    lg = kpool.tile([P, D], F32, tag="lg")
                    nc.vector.tensor_tensor(out=lg[:], in0=A[:, 2 * D + 1:],
                                            in1=k2s[:], op=OP.mult)
                    lgh = kpool.tile([P, NH], F32, tag="lgh")
                    nc.vector.tensor_reduce(
                        out=lgh[:], in_=lg[:].rearrange("p (h d) -> p h d", d=DH),
                        axis=mybir.AxisListType.X, op=OP.add)

                    exm = kpool.tile([P, NH + D], F32, tag="exm")
                    nc.scalar.activation(out=exm[:, :NH], in_=lgh[:], func=AF.Exp)
                    vw = kpool.tile([P, D], F32, tag="vw")
                    nc.vector.tensor_scalar_mul(vw[:], v2[:], ew[:])
                    nc.vector.tensor_tensor(
                        out=exm[:, NH:].rearrange("p (h d) -> p h d", d=DH),
                        in0=vw[:].rearrange("p (h d) -> p h d", d=DH),
                        in1=exm[:, :NH][:, :, None].to_broadcast([P, NH, DH]),
                        op=OP.mult)

                    nc.tensor.matmul(out=agg[:], lhsT=MT[:], rhs=exm[:],
                                     start=(ci == 0), stop=(ci == NCH - 1),
                                     skip_group_check=True)

                # ---------- bucket epilogue ----------
                den = bpool.tile([P, NH], F32, tag="den")
                nc.vector.tensor_scalar_max(den[:], agg[:, :NH], 1e-30)
                rd = bpool.tile([P, NH], F32, tag="rd")
                nc.vector.reciprocal(rd[:], den[:])
                attn = bpool.tile([P, D], F32, tag="attn")
                nc.vector.tensor_tensor(
                    out=attn[:].rearrange("p (h d) -> p h d", d=DH),
                    in0=agg[:, NH:].rearrange("p (h d) -> p h d", d=DH),
                    in1=rd[:][:, :, None].to_broadcast([P, NH, DH]),
                    op=OP.mult)
                attnT = bpool.tile([P, P], F32, tag="attnT")
                transpose_to_sb(attn[:], attnT[:], P, P)

                f1_ps = ppool.tile([P, 2 * D], F32, tag="A", space="PSUM")
                nc.tensor.matmul(out=f1_ps[:, :D], lhsT=attnT[:], rhs=cs["nw1a"][:],
                                 start=True, stop=False)
                nc.tensor.matmul(out=f1_ps[:, :D], lhsT=hT[:], rhs=cs["nw1b"][:],
                                 start=False, stop=False)
                nc.tensor.matmul(out=f1_ps[:, :D], lhsT=ones1[:], rhs=cs["nb1"][:],
                                 start=False, stop=True)
                fz = bpool.tile([P, D], F32, tag="fz")
                ln_relu(f1_ps[:, :D], fz[:], "no")
                fzT = bpool.tile([P, P], F32, tag="fzT")
                transpose_to_sb(fz[:], fzT[:], P, P)
                f2_ps = ppool.tile([P, 2 * D], F32, tag="A", space="PSUM")
                nc.tensor.matmul(out=f2_ps[:, :D], lhsT=fzT[:], rhs=cs["nw2"][:],
                                 start=True, stop=False)
                nc.tensor.matmul(out=f2_ps[:, :D], lhsT=ones1[:], rhs=cs["nb2"][:],
                                 start=False, stop=True)
                outt = bpool.tile([P, D], F32, tag="outt")
                nc.vector.tensor_tensor(out=outt[:], in0=f2_ps[:, :D], in1=hlt[:],
                                        op=OP.add)
                nc.sync.dma_start(out=out_d[b * P:b * P + bs, :], in_=outt[:bs, :])
    nc.compile()
    return nc




def _build_general(LT, flags):
    NCH = LT // P  # chunks per bucket
    nc = bacc.Bacc("TRN2", target_bir_lowering=False, detect_race_conditions=False)

    h_d = nc.dram_tensor("h", [N, D], F32, kind="ExternalInput")
    hl_d = nc.dram_tensor("hl", [NPAD, D], F32, kind="ExternalInput")
    dstrel_d = nc.dram_tensor("dstrel", [NB * LT], F32, kind="ExternalInput")
    srci_d = nc.dram_tensor("srci", [NB * LT], I32, kind="ExternalInput")
    refx_d = nc.dram_tensor("refx", [NB * LT, R + EF], F32, kind="ExternalInput")
    cd = {}
    cshapes = {
        "wdst": [D, 2 * D], "wsrc": [D, 2 * D], "wref": [R + EF, 2 * D + 1],
        "cb1": [1, 2 * D], "qw1": [D, D], "qb1": [1, D], "qw2": [D, D],
        "qb2": [1, D], "kw2": [D, D], "kb2": [1, D], "vw2": [D, D],
        "vb2": [1, D], "nw1a": [D, D], "nw1b": [D, D], "nb1": [1, D], "nw2": [D, D],
        "nb2": [1, D], "iotar": [P, P],
    }
    for nm in ("hk", "hv", "hq", "no"):
        if flags[nm + "_gb"]:
            cshapes[nm + "_grep"] = [P, D]
            cshapes[nm + "_brep"] = [P, D]
    for k, s in cshapes.items():
        cd[k] = nc.dram_tensor(k, s, F32, kind="ExternalInput")
    out_d = nc.dram_tensor("out", [NPC, D], F32, kind="ExternalOutput")

    qscale = 1.0 / np.sqrt(DH)

    with tile.TileContext(nc) as tc:
        with (
            tc.tile_pool(name="cpool", bufs=1) as cpool,
            tc.tile_pool(name="bpool", bufs=2) as bpool,
            tc.tile_pool(name="kpool", bufs=3) as kpool,
            tc.tile_pool(name="spool", bufs=4) as spool,
            tc.tile_pool(name="psum", bufs=1, space="PSUM") as ppool,
        ):
            # ---- constants resident in SBUF ----
            cs = {}
            for k, s in cshapes.items():
                t = cpool.tile(s, F32, tag="c_" + k)
                nc.sync.dma_start(out=t[:], in_=cd[k][:, :])
                cs[k] = t
            ident = cpool.tile([P, P], F32, tag="ident")
            make_identity(nc, ident[:])
            ones1 = cpool.tile([1, P], F32, tag="ones1")
            nc.vector.memset(ones1[:], 1.0)
            epsc = cpool.tile([P, 1], F32, tag="epsc")
            nc.vector.memset(epsc[:], EPS)

            def ln_relu(x_psum, out_sb, pref):
                """out_sb = relu(layernorm(x_psum) * g + beta), per-partition stats."""
                scr = spool.tile([P, P], F32, tag="scr")
                s1 = spool.tile([P, 1], F32, tag="s1")
                nc.scalar.activation(out=scr[:], in_=x_psum, func=AF.Copy,
                                     accum_out=s1[:])
                scr2 = spool.tile([P, P], F32, tag="scr2")
                s2 = spool.tile([P, 1], F32, tag="s2")
                nc.scalar.activation(out=scr2[:], in_=x_psum, func=AF.Square,
                                     accum_out=s2[:])
                mu = spool.tile([P, 1], F32, tag="mu")
                nc.vector.tensor_scalar_mul(mu[:], s1[:], 1.0 / D)
                var = spool.tile([P, 1], F32, tag="var")
                nc.vector.tensor_scalar(out=var[:], in0=s2[:], scalar1=1.0 / D,
                                        scalar2=None, op0=OP.mult)
                mu2 = spool.tile([P, 1], F32, tag="mu2")
                nc.vector.tensor_tensor(out=mu2[:], in0=mu[:], in1=mu[:], op=OP.mult)
                nc.vector.tensor_tensor(out=var[:], in0=var[:], in1=mu2[:],
                                        op=OP.subtract)
                sd = spool.tile([P, 1], F32, tag="sd")
                nc.scalar.activation(out=sd[:], in_=var[:], func=AF.Sqrt, bias=epsc[:])
                rs = spool.tile([P, 1], F32, tag="rs")
                nc.vector.reciprocal(rs[:], sd[:])
                nc.vector.tensor_scalar(out=out_sb, in0=x_psum, scalar1=mu[:],
                                        scalar2=rs[:], op0=OP.subtract, op1=OP.mult)
                if flags[pref + "_gb"]:
                    nc.vector.tensor_tensor(out=out_sb, in0=out_sb,
                                            in1=cs[pref + "_grep"][:], op=OP.mult)
                    nc.vector.tensor_tensor(out=out_sb, in0=out_sb,
                                            in1=cs[pref + "_brep"][:], op=OP.add)
                nc.vector.tensor_scalar_max(out_sb, out_sb, 0.0)

            def transpose_to_sb(src_sb, out_sb, np_, nf):
                """PE-transpose src_sb[:np_, :nf] -> out_sb[:nf, :np_] via PSUM."""
                tp = ppool.tile([P, P], F32, tag="tp", space="PSUM")
                nc.tensor.transpose(out=tp[:nf, :np_], in_=src_sb, identity=ident[:])
                nc.scalar.activation(out=out_sb, in_=tp[:nf, :np_], func=AF.Copy)

            for b in range(NB):
                bs = min(P, NPC - b * P)
                # ---------- bucket precompute ----------
                hlt = bpool.tile([P, D], F32, tag="hlt")
                nc.sync.dma_start(out=hlt[:], in_=hl_d[b * P:(b + 1) * P, :])
                hT = bpool.tile([P, P], F32, tag="hT")
                transpose_to_sb(hlt[:], hT[:], P, P)

                Bd = bpool.tile([P, 2 * D + 1 + D], F32, tag="Bd")  # [128, 385]

                # hW_dst = h_tile @ W1_dst (+ b1)  -> Bd[:, 0:256]
                hw_ps = ppool.tile([P, 2 * D], F32, tag="A", space="PSUM")
                nc.tensor.matmul(out=hw_ps[:], lhsT=hT[:], rhs=cs["wdst"][:],
                                 start=True, stop=not flags["cb1_nz"])
                if flags["cb1_nz"]:
                    nc.tensor.matmul(out=hw_ps[:], lhsT=ones1[:], rhs=cs["cb1"][:],
                                     start=False, stop=True)
                nc.scalar.activation(out=Bd[:, :2 * D], in_=hw_ps[:], func=AF.Copy)
                nc.vector.memset(Bd[:, 2 * D:2 * D + 1], flags["ew_b"])

                # q = MLP_q(h_tile) * qscale -> Bd[:, 257:385]
                q1_ps = ppool.tile([P, 2 * D], F32, tag="A", space="PSUM")
                nc.tensor.matmul(out=q1_ps[:, :D], lhsT=hT[:], rhs=cs["qw1"][:],
                                 start=True, stop=False)
                nc.tensor.matmul(out=q1_ps[:, :D], lhsT=ones1[:], rhs=cs["qb1"][:],
                                 start=False, stop=True)
                qz = bpool.tile([P, D], F32, tag="qz")
                ln_relu(q1_ps[:, :D], qz[:], "hq")
                qzT = bpool.tile([P, P], F32, tag="qzT")
                transpose_to_sb(qz[:], qzT[:], P, P)
                q2_ps = ppool.tile([P, 2 * D], F32, tag="A", space="PSUM")
                nc.tensor.matmul(out=q2_ps[:, :D], lhsT=qzT[:], rhs=cs["qw2"][:],
                                 start=True, stop=False)
                nc.tensor.matmul(out=q2_ps[:, :D], lhsT=ones1[:], rhs=cs["qb2"][:],
                                 start=False, stop=True)
                nc.scalar.activation(out=Bd[:, 2 * D + 1:], in_=q2_ps[:, :D],
                                     func=AF.Copy, scale=qscale)

                agg = ppool.tile([P, NH + D], F32, tag="agg", space="PSUM")

                # ---------- edge chunks ----------
                for ci in range(NCH):
                    e0 = b * LT + ci * P
                    dcol = kpool.tile([P, 1], F32, tag="dcol")
                    nc.sync.dma_start(out=dcol[:], in_=dstrelf_d[e0:e0 + P, None])
                    scol = kpool.tile([P, 1], I32, tag="scol")
                    nc.sync.dma_start(out=scol[:], in_=srci_d[e0:e0 + P, None])
                    refx = kpool.tile([P, R + EF], F32, tag="refx")
                    nc.sync.dma_start(out=refx[:], in_=refx_d[e0:e0 + P, :])
                    hsrc = kpool.tile([P, D], F32, tag="hsrc")
                    nc.gpsimd.indirect_dma_start(
                        out=hsrc[:], out_offset=None, in_=h_d[:, :],
                        in_offset=bass.IndirectOffsetOnAxis(ap=scol[:, :1], axis=0))

                    MT = kpool.tile([P, P], F32, tag="MT")
                    nc.vector.tensor_scalar(out=MT[:], in0=cs["iotar"][:],
                                            scalar1=dcol[:], scalar2=None,
                                            op0=OP.is_equal)
                    Mn = kpool.tile([P, P], F32, tag="Mn")
                    transpose_to_sb(MT[:], Mn[:], P, P)
                    hsT = kpool.tile([P, P], F32, tag="hsT")
                    transpose_to_sb(hsrc[:], hsT[:], P, P)
                    refT = kpool.tile([R + EF, P], F32, tag="refT")
                    transpose_to_sb(refx[:], refT[:], P, R + EF)

                    A = ppool.tile([P, 2 * D + 1 + D], F32, tag="A", space="PSUM")
                    nc.tensor.matmul(out=A[:], lhsT=Mn[:], rhs=Bd[:],
                                     start=True, stop=False)
                    nc.tensor.matmul(out=A[:, :2 * D], lhsT=hsT[:], rhs=cs["wsrc"][:],
                                     start=False, stop=False)
                    nc.tensor.matmul(out=A[:, :2 * D + 1], lhsT=refT[:],
                                     rhs=cs["wref"][:], start=False, stop=True)

                    zk = kpool.tile([P, D], F32, tag="zk")
                    ln_relu(A[:, :D], zk[:], "hk")
                    zv = kpool.tile([P, D], F32, tag="zv")
                    ln_relu(A[:, D:2 * D], zv[:], "hv")
                    zkT = kpool.tile([P, P], F32, tag="zkT")
                    transpose_to_sb(zk[:], zkT[:], P, P)
                    zvT = kpool.tile([P, P], F32, tag="zvT")
                    transpose_to_sb(zv[:], zvT[:], P, P)

                    k2 = ppool.tile([P, D], F32, tag="k2", space="PSUM")
                    nc.tensor.matmul(out=k2[:], lhsT=zkT[:], rhs=cs["kw2"][:],
                                     start=True, stop=not flags["kb2_nz"])
                    if flags["kb2_nz"]:
                        nc.tensor.matmul(out=k2[:], lhsT=ones1[:], rhs=cs["kb2"][:],
                                         start=False, stop=True)
                    v2 = ppool.tile([P, D], F32, tag="v2", space="PSUM")
                    nc.tensor.matmul(out=v2[:], lhsT=zvT[:], rhs=cs["vw2"][:],
                                     start=True, stop=not flags["vb2_nz"])
                    if flags["vb2_nz"]:
                        nc.tensor.matmul(out=v2[:], lhsT=ones1[:], rhs=cs["vb2"][:],
                                         start=False, stop=True)

                    ew = kpool.tile([P, 1], F32, tag="ew")
                    nc.scalar.activation(out=ew[:], in_=A[:, 2 * D:2 * D + 1],
                                         func=AF.Sigmoid)
                    k2s = kpool.tile([P, D], F32, tag="k2s")
                    nc.scalar.activation(out=k2s[:], in_=k2[:], func=AF.Copy)
                    lg = kpool.tile([P, D], F32, tag="lg")
                    nc.vector.tensor_tensor(out=lg[:], in0=A[:, 2 * D + 1:],
                                            in1=k2s[:], op=OP.mult)
                    lgh = kpool.tile([P, NH], F32, tag="lgh")
                    nc.vector.tensor_reduce(
                        out=lgh[:], in_=lg[:].rearrange("p (h d) -> p h d", d=DH),
                        axis=mybir.AxisListType.X, op=OP.add)

                    exm = kpool.tile([P, NH + D], F32, tag="exm")
                    nc.scalar.activation(out=exm[:, :NH], in_=lgh[:], func=AF.Exp)
                    vw = kpool.tile([P, D], F32, tag="vw")
                    nc.vector.tensor_scalar_mul(vw[:], v2[:], ew[:])
                    nc.vector.tensor_tensor(
                        out=exm[:, NH:].rearrange("p (h d) -> p h d", d=DH),
                        in0=vw[:].rearrange("p (h d) -> p h d", d=DH),
                        in1=exm[:, :NH][:, :, None].to_broadcast([P, NH, DH]),
                        op=OP.mult)

                    nc.tensor.matmul(out=agg[:], lhsT=MT[:], rhs=exm[:],
                                     start=(ci == 0), stop=(ci == NCH - 1),
                                     skip_group_check=True)

                # ---------- bucket epilogue ----------
                den = bpool.tile([P, NH], F32, tag="den")
                nc.vector.tensor_scalar_max(den[:], agg[:, :NH], 1e-30)
                rd = bpool.tile([P, NH], F32, tag="rd")
                nc.vector.reciprocal(rd[:], den[:])
                attn = bpool.tile([P, D], F32, tag="attn")
                nc.vector.tensor_tensor(
                    out=attn[:].rearrange("p (h d) -> p h d", d=DH),
                    in0=agg[:, NH:].rearrange("p (h d) -> p h d", d=DH),
                    in1=rd[:][:, :, None].to_broadcast([P, NH, DH]),
                    op=OP.mult)
                attnT = bpool.tile([P, P], F32, tag="attnT")
                transpose_to_sb(attn[:], attnT[:], P, P)

                f1_ps = ppool.tile([P, 2 * D], F32, tag="A", space="PSUM")
                nc.tensor.matmul(out=f1_ps[:, :D], lhsT=attnT[:], rhs=cs["nw1a"][:],
                                 start=True, stop=False)
                nc.tensor.matmul(out=f1_ps[:, :D], lhsT=hT[:], rhs=cs["nw1b"][:],
                                 start=False, stop=False)
                nc.tensor.matmul(out=f1_ps[:, :D], lhsT=ones1[:], rhs=cs["nb1"][:],
                                 start=False, stop=True)
                fz = bpool.tile([P, D], F32, tag="fz")
                ln_relu(f1_ps[:, :D], fz[:], "no")
                fzT = bpool.tile([P, P], F32, tag="fzT")
                transpose_to_sb(fz[:], fzT[:], P, P)
                f2_ps = ppool.tile([P, 2 * D], F32, tag="A", space="PSUM")
                nc.tensor.matmul(out=f2_ps[:, :D], lhsT=fzT[:], rhs=cs["nw2"][:],
                                 start=True, stop=False)
                nc.tensor.matmul(out=f2_ps[:, :D], lhsT=ones1[:], rhs=cs["nb2"][:],
                                 start=False, stop=True)
                outt = bpool.tile([P, D], F32, tag="outt")
                nc.vector.tensor_tensor(out=outt[:], in0=f2_ps[:, :D], in1=hlt[:],
                                        op=OP.add)
                nc.sync.dma_start(out=out_d[b * P:b * P + bs, :], in_=outt[:bs, :])
    nc.compile()
    return nc


BF16 = mybir.dt.bfloat16




# revision 14
# speedup vs baseline: 1.1869x; 1.1869x over previous
"""Trainium2 Bass kernel for nn_BaseX2HAttLayer (GNN edge-softmax attention).

v2 strategy (per core, edges sorted by dst into 10 buckets of 128 dst nodes):
  - src features enter the kv MLP via a TRANSPOSED batched dma_gather
    (hsT[d, e] = h[src_e, d]) so the src projection is a plain matmul term --
    no per-chunk indirect DMA, no transposes, no DVE add.
  - kv1 (+ negated feature-means from extra weight columns + negated ew
    logit) is materialized in PSUM by 3 accumulating matmuls, then one Act
    copy lands it in SBUF where Pool (gpsimd) can do the relu/normalize.
  - LN variance via tensor_tensor_reduce (sum x^2) + batched mean^2
    correction; rsqrt via exp(-0.5*ln(var+eps)) keeping the Act table fixed
    on {exp,ln,copy,relu,square}.
  - membership matrices built bf16 (4x DVE mode); logits/exp/weighted-v as
    in the baseline 3-phase scheme but with paired (2-chunk) DVE ops.
"""

import sys

for _p in ("/opt/trn_rl_repo",):
    if _p not in sys.path:
        sys.path.insert(0, _p)

import numpy as np

import concourse.bass as bass
import concourse.bacc as bacc
import concourse.tile as tile
from concourse import mybir
from concourse.bass_utils import run_bass_kernel_spmd
from concourse.masks import make_identity

N, E, D = 10000, 320000, 128
R, EF, NH = 64, 4, 16
DH = D // NH
NCORES = 8
NPC = N // NCORES
P = 128
NB = (NPC + P - 1) // P
NPAD = NB * P
EPS = 1e-5
F32 = mybir.dt.float32
I32 = mybir.dt.int32
I16 = mybir.dt.int16
BF16 = mybir.dt.bfloat16
AF = mybir.ActivationFunctionType
OP = mybir.AluOpType

GMAX = 896          # max num_idxs per dma_gather piece (HW cap < 1024)

LAST_RESULTS = None


def _prep(inputs):
    import ml_dtypes
    bf16 = ml_dtypes.bfloat16

    h = np.ascontiguousarray(inputs["h"], dtype=np.float32)
    r_feat = np.ascontiguousarray(inputs["r_feat"], dtype=np.float32)
    edge_feat = np.ascontiguousarray(inputs["edge_feat"], dtype=np.float32)
    ei = np.asarray(inputs["edge_index"])
    src = ei[0].astype(np.int64)
    dst = ei[1].astype(np.int64)

    perm = np.argsort(dst, kind="stable")
    sdst = dst[perm]
    counts = np.bincount(dst, minlength=N)
    cum = np.zeros(N + 1, dtype=np.int64)
    np.cumsum(counts, out=cum[1:])

    bstarts = np.empty((NCORES, NB), dtype=np.int64)
    bends = np.empty((NCORES, NB), dtype=np.int64)
    for c in range(NCORES):
        for b in range(NB):
            s = c * NPC + b * P
            e = min(s + P, (c + 1) * NPC)
            bstarts[c, b], bends[c, b] = s, e
    bcounts = cum[bends] - cum[bstarts]
    LT = int(((bcounts.max() + P - 1) // P) * P)
    EC = NB * LT

    f = lambda x: np.ascontiguousarray(np.asarray(x), dtype=np.float32)
    flags = {"ew_b": float(np.asarray(inputs["ew_b"]).reshape(-1)[0])}
    for nm in ("hk", "hv", "hq", "no"):
        g = f(inputs[nm + "_g"])
        be = f(inputs[nm + "_beta"])
        flags[nm + "_gb"] = not (np.all(g == 1.0) and np.all(be == 0.0))
    cb1 = np.concatenate([f(inputs["hk_b1"]), f(inputs["hv_b1"])])
    flags["cb1_nz"] = bool(np.any(cb1 != 0))
    flags["kb2_nz"] = bool(np.any(f(inputs["hk_b2"]) != 0))
    flags["vb2_nz"] = bool(np.any(f(inputs["hv_b2"]) != 0))
    other_b_zero = all(not np.any(f(inputs[k]) != 0) for k in
                       ("hq_b1", "hq_b2", "no_b1", "no_b2"))
    flags["fast"] = (not any(flags[nm + "_gb"] for nm in ("hk", "hv", "hq", "no"))
                    and not flags["cb1_nz"] and not flags["kb2_nz"]
                    and not flags["vb2_nz"] and other_b_zero)
    if not flags["fast"]:
        return None, LT, flags

    hk_w1, hv_w1 = f(inputs["hk_w1"]), f(inputs["hv_w1"])
    # input row blocks of W1: [edge_feat 0:EF | r_feat EF:EF+R | h_dst | h_src]
    Wk_dst, Wv_dst = hk_w1[EF + R:EF + R + D], hv_w1[EF + R:EF + R + D]
    Wk_src, Wv_src = hk_w1[EF + R + D:], hv_w1[EF + R + D:]
    # ref rows in refxT order: [r_feat (R) ; edge_feat (EF)]
    Wk_ref = np.concatenate([hk_w1[EF:EF + R], hk_w1[:EF]], 0)
    Wv_ref = np.concatenate([hv_w1[EF:EF + R], hv_w1[:EF]], 0)
    ew_w = f(inputs["ew_w"])[:, 0]  # [R]

    def kvx(Wk, Wv, extra=None):
        # [Wk | Wv | -mean(Wk) | -mean(Wv) | (extra)]
        cols = [Wk, Wv, -Wk.mean(1, keepdims=True), -Wv.mean(1, keepdims=True)]
        if extra is not None:
            cols.append(extra)
        return np.concatenate(cols, 1).astype(bf16)

    wdstx = kvx(Wk_dst, Wv_dst)                       # [128, 258]
    wsrcx = kvx(Wk_src, Wv_src)                       # [128, 258]
    ewneg = np.zeros((R + EF, 1), dtype=np.float32)
    ewneg[:R, 0] = -ew_w
    wrefx = kvx(Wk_ref, Wv_ref, ewneg)                # [68, 259]

    qscale = 1.0 / np.sqrt(DH)
    consts = {
        "wdstx": wdstx, "wsrcx": wsrcx, "wrefx": wrefx,
        "qw1b": f(inputs["hq_w1"]).astype(bf16),
        "qw2b": f(inputs["hq_w2"]).astype(bf16),
        "kw2b": f(inputs["hk_w2"]).astype(bf16),
        "vw2b": f(inputs["hv_w2"]).astype(bf16),
        "nw1ab": f(inputs["no_w1"])[:D].astype(bf16),
        "nw1bb": f(inputs["no_w1"])[D:].astype(bf16),
        "nw2b": f(inputs["no_w2"]).astype(bf16),
        "iotar": np.tile(np.arange(P, dtype=np.float32), (P, 1)).astype(bf16),
        "iotac": np.arange(P, dtype=np.float32)[:, None],
        "hb": h.astype(bf16),                         # [N, 128] gather table
    }
    NCH = LT // P

    in_maps = []
    for c in range(NCORES):
        dstrel = np.full(EC, -1000.0, dtype=np.float32)
        srci = np.zeros(EC, dtype=np.int16)
        refxT = np.zeros((R + EF, EC), dtype=bf16)
        for b in range(NB):
            lo, hi = cum[bstarts[c, b]], cum[bends[c, b]]
            L = hi - lo
            o = b * LT
            pidx = perm[lo:hi]
            dstrel[o:o + L] = (sdst[lo:hi] - bstarts[c, b]).astype(np.float32)
            srci[o:o + L] = src[pidx].astype(np.int16)
            refxT[:R, o:o + L] = r_feat[pidx].T
            refxT[R:, o:o + L] = edge_feat[pidx].T
        # wrap16 idx tables at partitions 16..31, one [128, LT//16] per bucket
        srcw = np.zeros((NB, 128, LT // 16), dtype=np.int16)
        for b in range(NB):
            srcw[b, 16:32, :] = srci[b * LT:(b + 1) * LT].reshape(LT // 16, 16).T
        hl = np.zeros((NPAD, D), dtype=np.float32)
        hl[:NPC] = h[c * NPC:(c + 1) * NPC]
        in_maps.append({
            "hl": hl,
            "dstrelb": dstrel.astype(bf16),
            "dstrelf": dstrel,
            "srcw": srcw,
            "refxT": refxT,
            **consts,
        })
    return in_maps, LT, flags


def _gpieces(LT):
    out, o = [], 0
    while o < LT:
        n = min(GMAX, LT - o)
        out.append((o, n))
        o += n
    return out


def _build_fast(LT, flags):
    NCH = LT // P
    NPAIR = NCH // 2
    assert NCH % 2 == 0
    nc = bacc.Bacc("TRN2", target_bir_lowering=False, detect_race_conditions=False)

    hl_d = nc.dram_tensor("hl", [NPAD, D], F32, kind="ExternalInput")
    hb_d = nc.dram_tensor("hb", [N, D], BF16, kind="ExternalInput")
    dstrelb_d = nc.dram_tensor("dstrelb", [NB * LT], BF16, kind="ExternalInput")
    dstrelf_d = nc.dram_tensor("dstrelf", [NB * LT], F32, kind="ExternalInput")
    srcw_d = nc.dram_tensor("srcw", [NB, 128, LT // 16], I16, kind="ExternalInput")
    refxT_d = nc.dram_tensor("refxT", [R + EF, NB * LT], BF16, kind="ExternalInput")
    cshapes = {
        "wdstx": ([D, 258], BF16), "wsrcx": ([D, 258], BF16),
        "wrefx": ([R + EF, 259], BF16),
        "qw1b": ([D, D], BF16), "qw2b": ([D, D], BF16),
        "kw2b": ([D, D], BF16), "vw2b": ([D, D], BF16),
        "nw1ab": ([D, D], BF16), "nw1bb": ([D, D], BF16), "nw2b": ([D, D], BF16),
        "iotar": ([P, P], BF16), "iotac": ([P, 1], F32),
    }
    cd = {k: nc.dram_tensor(k, sh, dt, kind="ExternalInput")
          for k, (sh, dt) in cshapes.items()}
    out_d = nc.dram_tensor("out", [NPC, D], F32, kind="ExternalOutput")
    import os as _os
    _DBG = bool(int(_os.environ.get("KDBG", "0")))
    if _DBG:
        dbg_kvq = nc.dram_tensor("dbg_kvq", [P, NCH * 387], F32, kind="ExternalOutput")
        dbg_lgall = nc.dram_tensor("dbg_lgall", [P, NCH * NH], F32, kind="ExternalOutput")
        dbg_v2all = nc.dram_tensor("dbg_v2all", [P, NCH * D], BF16, kind="ExternalOutput")
        dbg_stat = nc.dram_tensor("dbg_stat", [P, 2 * NCH], F32, kind="ExternalOutput")
        dbg_hsT = nc.dram_tensor("dbg_hsT", [P, LT], BF16, kind="ExternalOutput")
        dbg_agg = nc.dram_tensor("dbg_agg", [P, NH + D], F32, kind="ExternalOutput")
        dbg_exm = nc.dram_tensor("dbg_exm", [P, NH + D], BF16, kind="ExternalOutput")
        dbg_rsew = nc.dram_tensor("dbg_rsew", [P, NCH], F32, kind="ExternalOutput")
        dbg_exall = nc.dram_tensor("dbg_exall", [P, NCH * NH], BF16, kind="ExternalOutput")

    qscale = 1.0 / np.sqrt(DH)
    ew_b = flags["ew_b"]
    AW = 387
    AWS = AW          # A cols: k 0:128 | v 128:256 | -mk 256 | -mv 257 | -ew 258 | q 259:387

    with tile.TileContext(nc) as tc:
        with (
            tc.tile_pool(name="cpool", bufs=1) as cpool,
            tc.tile_pool(name="bpool", bufs=2) as bpool,
            tc.tile_pool(name="kpool", bufs=6) as kpool,
            tc.tile_pool(name="spool", bufs=4) as spool,
            tc.tile_pool(name="psum", bufs=1, space="PSUM") as ppool,
        ):
            cs = {}
            for k, (sh, dt) in cshapes.items():
                t = cpool.tile(sh, dt, tag="c_" + k)
                nc.sync.dma_start(out=t[:], in_=cd[k][:, :])
                cs[k] = t
            identb = cpool.tile([P, P], BF16, tag="identb")
            make_identity(nc, identb[:])
            ident = cpool.tile([P, P], F32, tag="ident")
            make_identity(nc, ident[:])
            epsc = cpool.tile([P, 1], F32, tag="epsc")
            nc.vector.memset(epsc[:], EPS)

            for b in range(NB):
                bs = min(P, NPC - b * P)
                # ---------------- bucket precompute ----------------
                hlt = bpool.tile([P, D], F32, tag="hlt")
                nc.sync.dma_start(out=hlt[:], in_=hl_d[b * P:(b + 1) * P, :])
                tpq = ppool.tile([P, AW], F32, tag="A", bufs=3, space="PSUM")
                nc.tensor.transpose(out=tpq[:, :P], in_=hlt[:], identity=ident[:])
                hTb = bpool.tile([P, P], BF16, tag="hTb")
                nc.scalar.activation(out=hTb[:], in_=tpq[:, :P], func=AF.Copy)

                Bd = bpool.tile([P, AW], BF16, tag="Bd")
                hw_ps = ppool.tile([P, AW], F32, tag="A", bufs=3, space="PSUM")
                hw_ps = hw_ps[:, :258]
                nc.tensor.matmul(out=hw_ps[:], lhsT=hTb[:], rhs=cs["wdstx"][:],
                                 start=True, stop=True)
                nc.scalar.activation(out=Bd[:, :258], in_=hw_ps[:], func=AF.Copy)
                nc.vector.memset(Bd[:, 258:259], -ew_b)

                # q MLP -> Bd[:, 259:387]  (baseline bn_stats approach)
                q1_ps = ppool.tile([P, AW], F32, tag="A", bufs=3, space="PSUM")
                q1_ps = q1_ps[:, :P]
                nc.tensor.matmul(out=q1_ps, lhsT=hTb[:], rhs=cs["qw1b"][:],
                                 start=True, stop=True)
                bsq = spool.tile([P, 6], F32, tag="bsq")
                nc.vector.bn_stats(out=bsq[:], in_=q1_ps)
                agq = spool.tile([P, 2], F32, tag="agq")
                nc.vector.bn_aggr(out=agq[:], in_=bsq[:])
                zq = bpool.tile([P, D], BF16, tag="zq")
                nc.vector.tensor_scalar(out=zq[:], in0=q1_ps,
                                        scalar1=agq[:, 0:1], scalar2=0.0,
                                        op0=OP.subtract, op1=OP.max)
                tpz = ppool.tile([P, 4 * P], BF16, tag="tpkv", bufs=2, space="PSUM")
                nc.tensor.transpose(out=tpz[:, :P], in_=zq[:], identity=identb[:])
                zqT = bpool.tile([P, P], BF16, tag="zqT")
                nc.scalar.activation(out=zqT[:], in_=tpz[:, :P], func=AF.Copy)
                q2_ps = ppool.tile([P, AW], F32, tag="A", bufs=3, space="PSUM")
                q2_ps = q2_ps[:, :P]
                nc.tensor.matmul(out=q2_ps, lhsT=zqT[:], rhs=cs["qw2b"][:],
                                 start=True, stop=True)
                # rs_q = exp(-0.5*ln(var+eps)); table stays in {exp,ln,...}
                sdq = spool.tile([P, 1], F32, tag="sdq")
                nc.scalar.activation(out=sdq[:], in_=agq[:, 1:2], func=AF.Sqrt,
                                     bias=epsc[:])
                rsq = spool.tile([P, 1], F32, tag="rsq")
                nc.vector.reciprocal(rsq[:], sdq[:])
                nc.vector.tensor_scalar(out=Bd[:, 259:], in0=q2_ps,
                                        scalar1=rsq[:], scalar2=qscale,
                                        op0=OP.mult, op1=OP.mult)

                # ---------------- bucket-wide loads / builds ----------------
                dstrow = bpool.tile([P, LT], BF16, tag="dstrow", bufs=1)
                nc.sync.dma_start(
                    out=dstrow[:],
                    in_=dstrelb_d[None, b * LT:(b + 1) * LT].to_broadcast([P, LT]))
                MnAll = bpool.tile([P, LT], BF16, tag="MnAll")
                nc.vector.tensor_scalar(out=MnAll[:], in0=dstrow[:],
                                        scalar1=cs["iotac"][:], scalar2=None,
                                        op0=OP.is_equal)
                dcolB = bpool.tile([P, NCH], F32, tag="dcolB")
                nc.sync.dma_start(
                    out=dcolB[:],
                    in_=dstrelf_d[b * LT:(b + 1) * LT].rearrange("(c p) -> p c", p=P))
                srcw = bpool.tile([128, LT // 16], I16, tag="srcw")
                nc.sync.dma_start(out=srcw[:], in_=srcw_d[b, :, :])
                hsg = bpool.tile([P, LT], BF16, tag="hsg")
                for gi, (o, n) in enumerate(_gpieces(LT)):
                    nc.gpsimd.dma_gather(
                        out_ap=hsg[:, o:o + n].rearrange("p (j d) -> p j d", d=D),
                        in_ap=hb_d[:, :], idxs_ap=srcw[:, o // 16:(o + n) // 16],
                        num_idxs=n, num_idxs_reg=n, elem_size=D, transpose=False)
                # transpose all chunks' src rows up front: hsTAll[d, e]
                hsTAll = bpool.tile([P, LT], BF16, tag="hsTAll")
                for g in range(0, NCH, 4):
                    ng = min(4, NCH - g)
                    tpg = ppool.tile([P, 4 * P], BF16, tag="tpkv", bufs=2,
                                     space="PSUM")
                    for t in range(ng):
                        nc.tensor.transpose(
                            out=tpg[:, t * P:(t + 1) * P],
                            in_=hsg[:, (g + t) * P:(g + t + 1) * P],
                            identity=identb[:])
                    nc.vector.tensor_copy(
                        out=hsTAll[:, g * P:(g + ng) * P],
                        in_=tpg[:, :ng * P])
                refT = bpool.tile([R + EF, LT], BF16, tag="refT")
                nc.sync.dma_start(out=refT[:], in_=refxT_d[:, b * LT:(b + 1) * LT])

                # bucket stashes
                kvq = bpool.tile([P, NCH * AWS], BF16, tag="kvq", bufs=2)
                MTall = bpool.tile([P, NCH * P], BF16, tag="MTall")
                v2all = bpool.tile([P, NCH * D], BF16, tag="v2all")
                lgall = bpool.tile([P, NCH * NH], F32, tag="lgall")
                statkv = bpool.tile([P, 2 * NCH], F32, tag="statkv")
                sqscr = bpool.tile([P, D], BF16, tag="sqscr")

                # ---------------- phase A ----------------
                import os as _os
                _STOP = int(_os.environ.get("KSTOP", "99"))
                if _STOP < 1:
                    outt0 = bpool.tile([P, D], F32, tag="outt")
                    nc.vector.tensor_tensor(out=outt0[:], in0=hlt[:], in1=hlt[:],
                                            op=OP.add)
                    nc.sync.dma_start(out=out_d[b * P:b * P + bs, :],
                                      in_=outt0[:bs, :])
                    continue
                for ci in range(NCH):
                    e0 = ci * P
                    A = ppool.tile([P, AW], F32, tag="A", bufs=3, space="PSUM")
                    nc.tensor.matmul(out=A[:], lhsT=MnAll[:, e0:e0 + P],
                                     rhs=Bd[:], start=True, stop=False)
                    nc.tensor.matmul(out=A[:, :258], lhsT=hsTAll[:, e0:e0 + P],
                                     rhs=cs["wsrcx"][:], start=False, stop=False)
                    nc.tensor.matmul(out=A[:, :259], lhsT=refT[:, e0:e0 + P],
                                     rhs=cs["wrefx"][:], start=False, stop=True)
                    if ci % 4 == 0:
                        g4 = min(4, NCH - ci)
                        nc.vector.tensor_tensor(
                            out=MTall[:, e0:e0 + g4 * P].rearrange(
                                "p (c j) -> p c j", c=g4),
                            in0=cs["iotar"][:][:, None, :].to_broadcast(
                                [P, g4, P]),
                            in1=dcolB[:, ci:ci + g4][:, :, None].to_broadcast(
                                [P, g4, P]),
                            op=OP.is_equal)
                    # land kv1 (k|v|negmk|negmv|negew|q) in SBUF
                    kv = kvq[:, ci * AWS:(ci + 1) * AWS]
                    nc.scalar.activation(out=kv, in_=A[:], func=AF.Copy)
                    kcol = kvq[:, ci * AWS + 256:ci * AWS + 257]
                    vcol = kvq[:, ci * AWS + 257:ci * AWS + 258]
                    # centered sum of squares per half: Square(x + negmu)
                    nc.scalar.activation(out=sqscr[:], in_=kv[:, :D],
                                         func=AF.Square, bias=kcol,
                                         accum_out=statkv[:, 2 * ci:2 * ci + 1])
                    nc.scalar.activation(out=sqscr[:], in_=kv[:, D:256],
                                         func=AF.Square, bias=vcol,
                                         accum_out=statkv[:, 2 * ci + 1:2 * ci + 2])
                    # relu(x - mu): TT add with broadcast negmu, then relu
                    zkv = kpool.tile([P, 2 * D], BF16, tag="zkv")
                    nc.vector.tensor_tensor(
                        out=zkv[:].rearrange("p (t d) -> p t d", t=2),
                        in0=kv[:, :256].rearrange("p (t d) -> p t d", t=2),
                        in1=kv[:, 256:258].rearrange(
                            "p (t o) -> p t o", t=2).to_broadcast([P, 2, D]),
                        op=OP.add)
                    nc.scalar.activation(out=zkv[:], in_=zkv[:], func=AF.Relu)
                    j = ci % 2
                    if j == 0:
                        tpkv = ppool.tile([P, 4 * P], BF16, tag="tpkv", bufs=2,
                                          space="PSUM")
                        kv2 = ppool.tile([P, 4 * D], F32, tag="kv2", bufs=2,
                                         space="PSUM")
                        zkvT = kpool.tile([P, 4 * P], BF16, tag="zkvT")
                    nc.tensor.transpose(out=tpkv[:, 2 * j * P:(2 * j + 1) * P],
                                        in_=zkv[:, :D], identity=identb[:])
                    nc.tensor.transpose(out=tpkv[:, (2 * j + 1) * P:(2 * j + 2) * P],
                                        in_=zkv[:, D:], identity=identb[:])
                    if j == 1 or ci == NCH - 1:
                        w = 2 * P * (j + 1)
                        nc.vector.tensor_copy(out=zkvT[:, :w], in_=tpkv[:, :w])
                        for t in range(j + 1):
                            nc.tensor.matmul(
                                out=kv2[:, 2 * t * D:(2 * t + 1) * D],
                                lhsT=zkvT[:, 2 * t * P:(2 * t + 1) * P],
                                rhs=cs["kw2b"][:], start=True, stop=True)
                            nc.tensor.matmul(
                                out=kv2[:, (2 * t + 1) * D:(2 * t + 2) * D],
                                lhsT=zkvT[:, (2 * t + 1) * P:(2 * t + 2) * P],
                                rhs=cs["vw2b"][:], start=True, stop=True)
                        nch = j + 1
                        c0 = ci - j
                        # logits: q (SBUF f32) * k2 (PSUM) -> reduce per head
                        lgscr = kpool.tile([P, 2 * D], F32, tag="lgscr")
                        nc.vector.tensor_tensor(
                            out=lgscr[:, :nch * D].rearrange(
                                "p (c d) -> p c d", c=nch),
                            in0=kvq[:, c0 * AWS:(c0 + nch) * AWS].rearrange(
                                "p (c w) -> p c w", c=nch)[:, :, 259:387],
                            in1=kv2[:, :nch * 2 * D].rearrange(
                                "p (c d) -> p c d", c=nch)[:, :, :D],
                            op=OP.mult)
                        nc.vector.tensor_reduce(
                            out=lgall[:, c0 * NH:(c0 + nch) * NH],
                            in_=lgscr[:, :nch * D].rearrange(
                                "p (c h d) -> p c h d", c=nch, d=DH),
                            axis=mybir.AxisListType.X, op=OP.add)
                        nc.scalar.activation(
                            out=v2all[:, c0 * D:(c0 + nch) * D].rearrange(
                                "p (c d) -> p c d", c=nch),
                            in_=kv2[:, :nch * 2 * D].rearrange(
                                "p (c d) -> p c d", c=nch)[:, :, D:],
                            func=AF.Copy)

                # ---------------- phase B (batched scalars) ----------------
                if _STOP < 2:
                    outt0 = bpool.tile([P, D], F32, tag="outt")
                    nc.vector.tensor_tensor(out=outt0[:], in0=hlt[:], in1=hlt[:],
                                            op=OP.add)
                    nc.sync.dma_start(out=out_d[b * P:b * P + bs, :],
                                      in_=outt0[:bs, :])
                    continue
                varkv = bpool.tile([P, 2 * NCH], F32, tag="varkv")
                nc.vector.tensor_scalar(out=varkv[:], in0=statkv[:],
                                        scalar1=1.0 / D, scalar2=None, op0=OP.mult)
                sdall = bpool.tile([P, 2 * NCH], F32, tag="sdall")
                nc.scalar.activation(out=sdall[:], in_=varkv[:], func=AF.Sqrt,
                                     bias=epsc[:])
                rsall = bpool.tile([P, 2 * NCH], F32, tag="rsall")
                nc.vector.reciprocal(rsall[:], sdall[:])
                # sigmoid(ew) from negated logit: 1/(1+exp(negew))
                ewx = bpool.tile([P, NCH], F32, tag="ewx")
                nc.scalar.activation(
                    out=ewx[:][:, :, None],
                    in_=kvq[:].rearrange("p (c w) -> p c w", w=AWS)[:, :, 258:259],
                    func=AF.Exp)
                nc.vector.tensor_scalar(out=ewx[:], in0=ewx[:], scalar1=1.0,
                                        scalar2=None, op0=OP.add)
                nc.vector.reciprocal(ewx[:], ewx[:])
                rsew = bpool.tile([P, NCH], F32, tag="rsew")
                nc.vector.tensor_tensor(out=rsew[:], in0=ewx[:],
                                        in1=rsall[:, 1::2], op=OP.mult)
                nc.vector.tensor_tensor(
                    out=lgall[:].rearrange("p (c h) -> p c h", h=NH),
                    in0=lgall[:].rearrange("p (c h) -> p c h", h=NH),
                    in1=rsall[:, 0::2][:, :, None].to_broadcast([P, NCH, NH]),
                    op=OP.mult)
                exall = bpool.tile([P, NCH * NH], BF16, tag="exall")
                half = (NCH // 2) * NH
                nc.scalar.activation(out=exall[:, :half], in_=lgall[:, :half],
                                     func=AF.Exp)
                nc.scalar.activation(out=exall[:, half:], in_=lgall[:, half:],
                                     func=AF.Exp)
                wgtvA = bpool.tile([P, NCH * NH], BF16, tag="wgtvA")
                nc.vector.tensor_tensor(
                    out=wgtvA[:].rearrange("p (c h) -> p c h", h=NH),
                    in0=exall[:].rearrange("p (c h) -> p c h", h=NH),
                    in1=rsew[:][:, :, None].to_broadcast([P, NCH, NH]),
                    op=OP.mult)

                # ---------------- phase C ----------------
                if _STOP < 3:
                    outt0 = bpool.tile([P, D], F32, tag="outt")
                    nc.vector.tensor_tensor(out=outt0[:], in0=hlt[:], in1=hlt[:],
                                            op=OP.add)
                    nc.sync.dma_start(out=out_d[b * P:b * P + bs, :],
                                      in_=outt0[:bs, :])
                    continue
                agg = ppool.tile([P, NH + D], F32, tag="agg", bufs=1, space="PSUM")
                for pi in range(NCH // 2):
                    exm = kpool.tile([P, 2 * (NH + D)], BF16, tag="exm")
                    c0 = 2 * pi
                    nc.vector.tensor_copy(
                        out=exm[:].rearrange("p (c f) -> p c f", c=2)[:, :, :NH],
                        in_=exall[:, c0 * NH:(c0 + 2) * NH].rearrange(
                            "p (c h) -> p c h", c=2))
                    nc.vector.tensor_tensor(
                        out=exm[:].rearrange(
                            "p (c f) -> p c f", c=2)[:, :, NH:].rearrange(
                            "p c (h d) -> p c h d", d=DH),
                        in0=v2all[:, c0 * D:(c0 + 2) * D].rearrange(
                            "p (c h d) -> p c h d", c=2, d=DH),
                        in1=wgtvA[:, c0 * NH:(c0 + 2) * NH].rearrange(
                            "p (c h) -> p c h", c=2)[:, :, :, None].to_broadcast(
                            [P, 2, NH, DH]),
                        op=OP.mult)
                    if _DBG and b == 0 and pi == 0:
                        nc.sync.dma_start(out=dbg_exm[:, :], in_=exm[:, :NH + D])
                        nc.sync.dma_start(out=dbg_rsew[:, :], in_=rsew[:])
                        nc.sync.dma_start(out=dbg_exall[:, :], in_=exall[:])
                    for t in range(2):
                        ci = c0 + t
                        nc.tensor.matmul(
                            out=agg[:], lhsT=MTall[:, ci * P:(ci + 1) * P],
                            rhs=exm[:, t * (NH + D):(t + 1) * (NH + D)],
                            start=(ci == 0), stop=(ci == NCH - 1),
                            skip_group_check=True)

                # ---------------- bucket epilogue ----------------
                den = bpool.tile([P, NH], F32, tag="den")
                nc.vector.tensor_scalar_max(den[:], agg[:, :NH], 1e-30)
                rd = bpool.tile([P, NH], F32, tag="rd")
                nc.vector.reciprocal(rd[:], den[:])
                attn = bpool.tile([P, D], F32, tag="attn")
                nc.vector.tensor_tensor(
                    out=attn[:].rearrange("p (h d) -> p h d", d=DH),
                    in0=agg[:, NH:].rearrange("p (h d) -> p h d", d=DH),
                    in1=rd[:][:, :, None].to_broadcast([P, NH, DH]),
                    op=OP.mult)
                tpa = ppool.tile([P, AW], F32, tag="A", bufs=3, space="PSUM")
                nc.tensor.transpose(out=tpa[:, :P], in_=attn[:], identity=ident[:])
                attnT = bpool.tile([P, P], BF16, tag="attnT")
                nc.scalar.activation(out=attnT[:], in_=tpa[:, :P], func=AF.Copy)

                f1_ps = ppool.tile([P, AW], F32, tag="A", bufs=3, space="PSUM")
                f1_ps = f1_ps[:, :P]
                nc.tensor.matmul(out=f1_ps, lhsT=attnT[:], rhs=cs["nw1ab"][:],
                                 start=True, stop=False)
                nc.tensor.matmul(out=f1_ps, lhsT=hTb[:], rhs=cs["nw1bb"][:],
                                 start=False, stop=True)
                bsf = spool.tile([P, 6], F32, tag="bsf")
                nc.vector.bn_stats(out=bsf[:], in_=f1_ps)
                agf = spool.tile([P, 2], F32, tag="agf")
                nc.vector.bn_aggr(out=agf[:], in_=bsf[:])
                zf = bpool.tile([P, D], BF16, tag="zf")
                nc.vector.tensor_scalar(out=zf[:], in0=f1_ps,
                                        scalar1=agf[:, 0:1], scalar2=0.0,
                                        op0=OP.subtract, op1=OP.max)
                tpf2 = ppool.tile([P, 4 * P], BF16, tag="tpkv", bufs=2, space="PSUM")
                nc.tensor.transpose(out=tpf2[:, :P], in_=zf[:], identity=identb[:])
                fzT = bpool.tile([P, P], BF16, tag="fzT")
                nc.scalar.activation(out=fzT[:], in_=tpf2[:, :P], func=AF.Copy)
                f2_ps = ppool.tile([P, AW], F32, tag="A", bufs=3, space="PSUM")
                f2_ps = f2_ps[:, :P]
                nc.tensor.matmul(out=f2_ps, lhsT=fzT[:], rhs=cs["nw2b"][:],
                                 start=True, stop=True)
                sdf = spool.tile([P, 1], F32, tag="sdf")
                nc.scalar.activation(out=sdf[:], in_=agf[:, 1:2], func=AF.Sqrt,
                                     bias=epsc[:])
                rsf = spool.tile([P, 1], F32, tag="rsf")
                nc.vector.reciprocal(rsf[:], sdf[:])
                t1 = bpool.tile([P, D], F32, tag="t1")
                nc.vector.tensor_scalar(out=t1[:], in0=f2_ps,
                                        scalar1=rsf[:], scalar2=None, op0=OP.mult)
                outt = bpool.tile([P, D], F32, tag="outt")
                nc.vector.tensor_tensor(out=outt[:], in0=t1[:], in1=hlt[:],
                                        op=OP.add)
                nc.sync.dma_start(out=out_d[b * P:b * P + bs, :], in_=outt[:bs, :])
    nc.compile()
    return nc


def kernel(**inputs):
    global LAST_RESULTS
    in_maps, LT, flags = _prep(inputs)
    if in_maps is None:
        in_maps_g, LTg, flags_g = _prep_general(inputs)
        nc = _build_general(LTg, flags_g)
        import os
        trace = bool(int(os.environ.get("KBENCH_TRACE", "0")))
        res = run_bass_kernel_spmd(nc, in_maps_g, core_ids=list(range(NCORES)),
                                   trace=trace)
        LAST_RESULTS = res
        outs = res.results
        return np.concatenate([outs[c]["out"] for c in range(NCORES)],
                              axis=0).astype(np.float32)
    nc = _build_fast(LT, flags)
    import os
    trace = bool(int(os.environ.get("KBENCH_TRACE", "0")))
    res = run_bass_kernel_spmd(nc, in_maps, core_ids=list(range(NCORES)),
                               trace=trace)
    LAST_RESULTS = res
    outs = res.results
    full = np.concatenate([outs[c]["out"] for c in range(NCORES)], axis=0)
    return full.astype(np.float32)


def _prep_general(inputs):
    h = np.ascontiguousarray(inputs["h"], dtype=np.float32)
    r_feat = np.ascontiguousarray(inputs["r_feat"], dtype=np.float32)
    edge_feat = np.ascontiguousarray(inputs["edge_feat"], dtype=np.float32)
    ei = np.asarray(inputs["edge_index"])
    src = ei[0].astype(np.int64)
    dst = ei[1].astype(np.int64)

    perm = np.argsort(dst, kind="stable")
    sdst = dst[perm]
    counts = np.bincount(dst, minlength=N)
    cum = np.zeros(N + 1, dtype=np.int64)
    np.cumsum(counts, out=cum[1:])

    # bucket (core c, bucket b) covers global nodes [c*NPC + b*P, min(..+P, (c+1)*NPC))
    bstarts = np.empty((NCORES, NB), dtype=np.int64)
    bends = np.empty((NCORES, NB), dtype=np.int64)
    for c in range(NCORES):
        for b in range(NB):
            s = c * NPC + b * P
            e = min(s + P, (c + 1) * NPC)
            bstarts[c, b], bends[c, b] = s, e
    bcounts = cum[bends] - cum[bstarts]
    LT = int(((bcounts.max() + P - 1) // P) * P)
    EC = NB * LT

    in_maps = []
    for c in range(NCORES):
        dstrel = np.full(EC, -1000.0, dtype=np.float32)
        srci = np.zeros(EC, dtype=np.int32)
        refx = np.zeros((EC, R + EF), dtype=np.float32)
        for b in range(NB):
            lo, hi = cum[bstarts[c, b]], cum[bends[c, b]]
            L = hi - lo
            o = b * LT
            pidx = perm[lo:hi]
            dstrel[o:o + L] = (sdst[lo:hi] - bstarts[c, b]).astype(np.float32)
            srci[o:o + L] = src[pidx].astype(np.int32)
            refx[o:o + L, :R] = r_feat[pidx]
            refx[o:o + L, R:] = edge_feat[pidx]
        hl = np.zeros((NPAD, D), dtype=np.float32)
        hl[:NPC] = h[c * NPC:(c + 1) * NPC]
        in_maps.append({
            "h": h, "hl": hl, "dstrel": dstrel, "srci": srci, "refx": refx,
        })

    f = lambda x: np.ascontiguousarray(np.asarray(x), dtype=np.float32)
    hk_w1, hv_w1 = f(inputs["hk_w1"]), f(inputs["hv_w1"])
    wdst = np.concatenate([hk_w1[EF + R:EF + R + D], hv_w1[EF + R:EF + R + D]], 1)
    wsrc = np.concatenate([hk_w1[EF + R + D:], hv_w1[EF + R + D:]], 1)
    wref = np.zeros((R + EF, 2 * D + 1), dtype=np.float32)
    wref[:R, :D] = hk_w1[EF:EF + R]
    wref[:R, D:2 * D] = hv_w1[EF:EF + R]
    wref[R:, :D] = hk_w1[:EF]
    wref[R:, D:2 * D] = hv_w1[:EF]
    wref[:R, 2 * D] = f(inputs["ew_w"])[:, 0]
    cb1 = np.concatenate([f(inputs["hk_b1"]), f(inputs["hv_b1"])])[None, :]  # [1,256]
    ew_b = float(np.asarray(inputs["ew_b"]).reshape(-1)[0])

    consts = {
        "wdst": wdst, "wsrc": wsrc, "wref": wref, "cb1": cb1,
        "qw1": f(inputs["hq_w1"]), "qb1": f(inputs["hq_b1"])[None, :],
        "qw2": f(inputs["hq_w2"]), "qb2": f(inputs["hq_b2"])[None, :],
        "kw2": f(inputs["hk_w2"]), "kb2": f(inputs["hk_b2"])[None, :],
        "vw2": f(inputs["hv_w2"]), "vb2": f(inputs["hv_b2"])[None, :],
        "nw1a": f(inputs["no_w1"])[:D], "nw1b": f(inputs["no_w1"])[D:],
        "nb1": f(inputs["no_b1"])[None, :],
        "nw2": f(inputs["no_w2"]), "nb2": f(inputs["no_b2"])[None, :],
        "iotar": np.tile(np.arange(P, dtype=np.float32), (P, 1)),
    }
    gb = {}
    flags = {"ew_b": ew_b}
    for nm in ("hk", "hv", "hq", "no"):
        g = f(inputs[nm + "_g"])
        be = f(inputs[nm + "_beta"])
        trivial = bool(np.all(g == 1.0) and np.all(be == 0.0))
        flags[nm + "_gb"] = not trivial
        if not trivial:
            gb[nm + "_grep"] = np.tile(g[None, :], (P, 1))
            gb[nm + "_brep"] = np.tile(be[None, :], (P, 1))
    flags["cb1_nz"] = bool(np.any(cb1 != 0))
    flags["kb2_nz"] = bool(np.any(consts["kb2"] != 0))
    flags["vb2_nz"] = bool(np.any(consts["vb2"] != 0))
    other_b_zero = all(not np.any(consts[k] != 0) for k in
                       ("qb1", "qb2", "nb1", "nb2"))
    flags["fast"] = (not any(flags[nm + "_gb"] for nm in ("hk", "hv", "hq", "no"))
                     and not flags["cb1_nz"] and not flags["kb2_nz"]
                     and not flags["vb2_nz"] and other_b_zero)
    consts.update(gb)
    if not flags["fast"]:
        for m in in_maps:
            m.update(consts)
        return in_maps, LT, flags

    # ---- fast path arrays (bf16 matmul operands, pre-transposed/pre-projected) ----
    import ml_dtypes
    bf16 = ml_dtypes.bfloat16
    NCH = LT // P
    hsw = (h @ wsrc).astype(bf16)                       # [N, 256] src projection table
    fc = {
        "hsw": hsw,
        "wdstb": wdst.astype(bf16),
        "wrefb": wref.astype(bf16),
        "qw1b": consts["qw1"].astype(bf16), "qw2b": consts["qw2"].astype(bf16),
        "kw2b": consts["kw2"].astype(bf16), "vw2b": consts["vw2"].astype(bf16),
        "nw1ab": consts["nw1a"].astype(bf16), "nw1bb": consts["nw1b"].astype(bf16),
        "nw2b": consts["nw2"].astype(bf16),
        "iotar": consts["iotar"],
        "iotac": np.arange(P, dtype=np.float32)[:, None],
    }
    fast_maps = []
    for c, m in enumerate(in_maps):
        refxT = np.zeros((NB * NCH, R + EF, P), dtype=bf16)
        rx = m["refx"].reshape(NB * NCH, P, R + EF)
        refxT[:] = rx.transpose(0, 2, 1).astype(bf16)
        fast_maps.append({
            "hl": m["hl"],
            "dstrelb": m["dstrel"].astype(bf16),
            "dstrelf": m["dstrel"],
            "srci": m["srci"],
            "refxT": refxT,
            **fc,
        })
    return fast_maps, LT, flags


def _build_general(LT, flags):
    NCH = LT // P  # chunks per bucket
    nc = bacc.Bacc("TRN2", target_bir_lowering=False, detect_race_conditions=False)

    h_d = nc.dram_tensor("h", [N, D], F32, kind="ExternalInput")
    hl_d = nc.dram_tensor("hl", [NPAD, D], F32, kind="ExternalInput")
    dstrel_d = nc.dram_tensor("dstrel", [NB * LT], F32, kind="ExternalInput")
    srci_d = nc.dram_tensor("srci", [NB * LT], I32, kind="ExternalInput")
    refx_d = nc.dram_tensor("refx", [NB * LT, R + EF], F32, kind="ExternalInput")
    cd = {}
    cshapes = {
        "wdst": [D, 2 * D], "wsrc": [D, 2 * D], "wref": [R + EF, 2 * D + 1],
        "cb1": [1, 2 * D], "qw1": [D, D], "qb1": [1, D], "qw2": [D, D],
        "qb2": [1, D], "kw2": [D, D], "kb2": [1, D], "vw2": [D, D],
        "vb2": [1, D], "nw1a": [D, D], "nw1b": [D, D], "nb1": [1, D], "nw2": [D, D],
        "nb2": [1, D], "iotar": [P, P],
    }
    for nm in ("hk", "hv", "hq", "no"):
        if flags[nm + "_gb"]:
            cshapes[nm + "_grep"] = [P, D]
            cshapes[nm + "_brep"] = [P, D]
    for k, s in cshapes.items():
        cd[k] = nc.dram_tensor(k, s, F32, kind="ExternalInput")
    out_d = nc.dram_tensor("out", [NPC, D], F32, kind="ExternalOutput")

    qscale = 1.0 / np.sqrt(DH)

    with tile.TileContext(nc) as tc:
        with (
            tc.tile_pool(name="cpool", bufs=1) as cpool,
            tc.tile_pool(name="bpool", bufs=2) as bpool,
            tc.tile_pool(name="kpool", bufs=3) as kpool,
            tc.tile_pool(name="spool", bufs=4) as spool,
            tc.tile_pool(name="psum", bufs=1, space="PSUM") as ppool,
        ):
            # ---- constants resident in SBUF ----
            cs = {}
            for k, s in cshapes.items():
                t = cpool.tile(s, F32, tag="c_" + k)
                nc.sync.dma_start(out=t[:], in_=cd[k][:, :])
                cs[k] = t
            ident = cpool.tile([P, P], F32, tag="ident")
            make_identity(nc, ident[:])
            ones1 = cpool.tile([1, P], F32, tag="ones1")
            nc.vector.memset(ones1[:], 1.0)
            epsc = cpool.tile([P, 1], F32, tag="epsc")
            nc.vector.memset(epsc[:], EPS)

            def ln_relu(x_psum, out_sb, pref):
                """out_sb = relu(layernorm(x_psum) * g + beta), per-partition stats."""
                scr = spool.tile([P, P], F32, tag="scr")
                s1 = spool.tile([P, 1], F32, tag="s1")
                nc.scalar.activation(out=scr[:], in_=x_psum, func=AF.Copy,
                                     accum_out=s1[:])
                scr2 = spool.tile([P, P], F32, tag="scr2")
                s2 = spool.tile([P, 1], F32, tag="s2")
                nc.scalar.activation(out=scr2[:], in_=x_psum, func=AF.Square,
                                     accum_out=s2[:])
                mu = spool.tile([P, 1], F32, tag="mu")
                nc.vector.tensor_scalar_mul(mu[:], s1[:], 1.0 / D)
                var = spool.tile([P, 1], F32, tag="var")
                nc.vector.tensor_scalar(out=var[:], in0=s2[:], scalar1=1.0 / D,
                                        scalar2=None, op0=OP.mult)
                mu2 = spool.tile([P, 1], F32, tag="mu2")
                nc.vector.tensor_tensor(out=mu2[:], in0=mu[:], in1=mu[:], op=OP.mult)
                nc.vector.tensor_tensor(out=var[:], in0=var[:], in1=mu2[:],
                                        op=OP.subtract)
                sd = spool.tile([P, 1], F32, tag="sd")
                nc.scalar.activation(out=sd[:], in_=var[:], func=AF.Sqrt, bias=epsc[:])
                rs = spool.tile([P, 1], F32, tag="rs")
                nc.vector.reciprocal(rs[:], sd[:])
                nc.vector.tensor_scalar(out=out_sb, in0=x_psum, scalar1=mu[:],
                                        scalar2=rs[:], op0=OP.subtract, op1=OP.mult)
                if flags[pref + "_gb"]:
                    nc.vector.tensor_tensor(out=out_sb, in0=out_sb,
                                            in1=cs[pref + "_grep"][:], op=OP.mult)
                    nc.vector.tensor_tensor(out=out_sb, in0=out_sb,
                                            in1=cs[pref + "_brep"][:], op=OP.add)
                nc.vector.tensor_scalar_max(out_sb, out_sb, 0.0)

            def transpose_to_sb(src_sb, out_sb, np_, nf):
                """PE-transpose src_sb[:np_, :nf] -> out_sb[:nf, :np_] via PSUM."""
                tp = ppool.tile([P, P], F32, tag="tp", space="PSUM")
                nc.tensor.transpose(out=tp[:nf, :np_], in_=src_sb, identity=ident[:])
                nc.scalar.activation(out=out_sb, in_=tp[:nf, :np_], func=AF.Copy)

            for b in range(NB):
                bs = min(P, NPC - b * P)
                # ---------- bucket precompute ----------
                hlt = bpool.tile([P, D], F32, tag="hlt")
                nc.sync.dma_start(out=hlt[:], in_=hl_d[b * P:(b + 1) * P, :])
                hT = bpool.tile([P, P], F32, tag="hT")
                transpose_to_sb(hlt[:], hT[:], P, P)

                Bd = bpool.tile([P, 2 * D + 1 + D], F32, tag="Bd")  # [128, 385]

                # hW_dst = h_tile @ W1_dst (+ b1)  -> Bd[:, 0:256]
                hw_ps = ppool.tile([P, 2 * D], F32, tag="A", space="PSUM")
                nc.tensor.matmul(out=hw_ps[:], lhsT=hT[:], rhs=cs["wdst"][:],
                                 start=True, stop=not flags["cb1_nz"])
                if flags["cb1_nz"]:
                    nc.tensor.matmul(out=hw_ps[:], lhsT=ones1[:], rhs=cs["cb1"][:],
                                     start=False, stop=True)
                nc.scalar.activation(out=Bd[:, :2 * D], in_=hw_ps[:], func=AF.Copy)
                nc.vector.memset(Bd[:, 2 * D:2 * D + 1], flags["ew_b"])

                # q = MLP_q(h_tile) * qscale -> Bd[:, 257:385]
                q1_ps = ppool.tile([P, 2 * D], F32, tag="A", space="PSUM")
                nc.tensor.matmul(out=q1_ps[:, :D], lhsT=hT[:], rhs=cs["qw1"][:],
                                 start=True, stop=False)
                nc.tensor.matmul(out=q1_ps[:, :D], lhsT=ones1[:], rhs=cs["qb1"][:],
                                 start=False, stop=True)
                qz = bpool.tile([P, D], F32, tag="qz")
                ln_relu(q1_ps[:, :D], qz[:], "hq")
                qzT = bpool.tile([P, P], F32, tag="qzT")
                transpose_to_sb(qz[:], qzT[:], P, P)
                q2_ps = ppool.tile([P, 2 * D], F32, tag="A", space="PSUM")
                nc.tensor.matmul(out=q2_ps[:, :D], lhsT=qzT[:], rhs=cs["qw2"][:],
                                 start=True, stop=False)
                nc.tensor.matmul(out=q2_ps[:, :D], lhsT=ones1[:], rhs=cs["qb2"][:],
                                 start=False, stop=True)
                nc.scalar.activation(out=Bd[:, 2 * D + 1:], in_=q2_ps[:, :D],
                                     func=AF.Copy, scale=qscale)

                agg = ppool.tile([P, NH + D], F32, tag="agg", space="PSUM")

                # ---------- edge chunks ----------
                for ci in range(NCH):
                    e0 = b * LT + ci * P
                    dcol = kpool.tile([P, 1], F32, tag="dcol")
                    nc.sync.dma_start(out=dcol[:], in_=dstrelf_d[e0:e0 + P, None])
                    scol = kpool.tile([P, 1], I32, tag="scol")
                    nc.sync.dma_start(out=scol[:], in_=srci_d[e0:e0 + P, None])
                    refx = kpool.tile([P, R + EF], F32, tag="refx")
                    nc.sync.dma_start(out=refx[:], in_=refx_d[e0:e0 + P, :])
                    hsrc = kpool.tile([P, D], F32, tag="hsrc")
                    nc.gpsimd.indirect_dma_start(
                        out=hsrc[:], out_offset=None, in_=h_d[:, :],
                        in_offset=bass.IndirectOffsetOnAxis(ap=scol[:, :1], axis=0))

                    MT = kpool.tile([P, P], F32, tag="MT")
                    nc.vector.tensor_scalar(out=MT[:], in0=cs["iotar"][:],
                                            scalar1=dcol[:], scalar2=None,
                                            op0=OP.is_equal)
                    Mn = kpool.tile([P, P], F32, tag="Mn")
                    transpose_to_sb(MT[:], Mn[:], P, P)
                    hsT = kpool.tile([P, P], F32, tag="hsT")
                    transpose_to_sb(hsrc[:], hsT[:], P, P)
                    refT = kpool.tile([R + EF, P], F32, tag="refT")
                    transpose_to_sb(refx[:], refT[:], P, R + EF)

                    A = ppool.tile([P, 2 * D + 1 + D], F32, tag="A", space="PSUM")
                    nc.tensor.matmul(out=A[:], lhsT=Mn[:], rhs=Bd[:],
                                     start=True, stop=False)
                    nc.tensor.matmul(out=A[:, :2 * D], lhsT=hsT[:], rhs=cs["wsrc"][:],
                                     start=False, stop=False)
                    nc.tensor.matmul(out=A[:, :2 * D + 1], lhsT=refT[:],
                                     rhs=cs["wref"][:], start=False, stop=True)

                    zk = kpool.tile([P, D], F32, tag="zk")
                    ln_relu(A[:, :D], zk[:], "hk")
                    zv = kpool.tile([P, D], F32, tag="zv")
                    ln_relu(A[:, D:2 * D], zv[:], "hv")
                    zkT = kpool.tile([P, P], F32, tag="zkT")
                    transpose_to_sb(zk[:], zkT[:], P, P)
                    zvT = kpool.tile([P, P], F32, tag="zvT")
                    transpose_to_sb(zv[:], zvT[:], P, P)

                    k2 = ppool.tile([P, D], F32, tag="k2", space="PSUM")
                    nc.tensor.matmul(out=k2[:], lhsT=zkT[:], rhs=cs["kw2"][:],
                                     start=True, stop=not flags["kb2_nz"])
                    if flags["kb2_nz"]:
                        nc.tensor.matmul(out=k2[:], lhsT=ones1[:], rhs=cs["kb2"][:],
                                         start=False, stop=True)
                    v2 = ppool.tile([P, D], F32, tag="v2", space="PSUM")
                    nc.tensor.matmul(out=v2[:], lhsT=zvT[:], rhs=cs["vw2"][:],
                                     start=True, stop=not flags["vb2_nz"])
                    if flags["vb2_nz"]:
                        nc.tensor.matmul(out=v2[:], lhsT=ones1[:], rhs=cs["vb2"][:],
                                         start=False, stop=True)

                    ew = kpool.tile([P, 1], F32, tag="ew")
                    nc.scalar.activation(out=ew[:], in_=A[:, 2 * D:2 * D + 1],
                                         func=AF.Sigmoid)
                    k2s = kpool.tile([P, D], F32, tag="k2s")
                    nc.scalar.activation(out=k2s[:], in_=k2[:], func=AF.Copy)
                    lg = kpool.tile([P, D], F32, tag="lg")
                    nc.vector.tensor_tensor(out=lg[:], in0=A[:, 2 * D + 1:],
                                            in1=k2s[:], op=OP.mult)
                    lgh = kpool.tile([P, NH], F32, tag="lgh")
                    nc.vector.tensor_reduce(
                        out=lgh[:], in_=lg[:].rearrange("p (h d) -> p h d", d=DH),
                        axis=mybir.AxisListType.X, op=OP.add)

                    exm = kpool.tile([P, NH + D], F32, tag="exm")
                    nc.scalar.activation(out=exm[:, :NH], in_=lgh[:], func=AF.Exp)
                    vw = kpool.tile([P, D], F32, tag="vw")
                    nc.vector.tensor_scalar_mul(vw[:], v2[:], ew[:])
                    nc.vector.tensor_tensor(
                        out=exm[:, NH:].rearrange("p (h d) -> p h d", d=DH),
                        in0=vw[:].rearrange("p (h d) -> p h d", d=DH),
                        in1=exm[:, :NH][:, :, None].to_broadcast([P, NH, DH]),
                        op=OP.mult)

                    nc.tensor.matmul(out=agg[:], lhsT=MT[:], rhs=exm[:],
                                     start=(ci == 0), stop=(ci == NCH - 1),
                                     skip_group_check=True)

                # ---------- bucket epilogue ----------
                den = bpool.tile([P, NH], F32, tag="den")
                nc.vector.tensor_scalar_max(den[:], agg[:, :NH], 1e-30)
                rd = bpool.tile([P, NH], F32, tag="rd")
                nc.vector.reciprocal(rd[:], den[:])
                attn = bpool.tile([P, D], F32, tag="attn")
                nc.vector.tensor_tensor(
                    out=attn[:].rearrange("p (h d) -> p h d", d=DH),
                    in0=agg[:, NH:].rearrange("p (h d) -> p h d", d=DH),
                    in1=rd[:][:, :, None].to_broadcast([P, NH, DH]),
                    op=OP.mult)
                attnT = bpool.tile([P, P], F32, tag="attnT")
                transpose_to_sb(attn[:], attnT[:], P, P)

                f1_ps = ppool.tile([P, 2 * D], F32, tag="A", space="PSUM")
                nc.tensor.matmul(out=f1_ps[:, :D], lhsT=attnT[:], rhs=cs["nw1a"][:],
                                 start=True, stop=False)
                nc.tensor.matmul(out=f1_ps[:, :D], lhsT=hT[:], rhs=cs["nw1b"][:],
                                 start=False, stop=False)
                nc.tensor.matmul(out=f1_ps[:, :D], lhsT=ones1[:], rhs=cs["nb1"][:],
                                 start=False, stop=True)
                fz = bpool.tile([P, D], F32, tag="fz")
                ln_relu(f1_ps[:, :D], fz[:], "no")
                fzT = bpool.tile([P, P], F32, tag="fzT")
                transpose_to_sb(fz[:], fzT[:], P, P)
                f2_ps = ppool.tile([P, 2 * D], F32, tag="A", space="PSUM")
                nc.tensor.matmul(out=f2_ps[:, :D], lhsT=fzT[:], rhs=cs["nw2"][:],
                                 start=True, stop=False)
                nc.tensor.matmul(out=f2_ps[:, :D], lhsT=ones1[:], rhs=cs["nb2"][:],
                                 start=False, stop=True)
                outt = bpool.tile([P, D], F32, tag="outt")
                nc.vector.tensor_tensor(out=outt[:], in0=f2_ps[:, :D], in1=hlt[:],
                                        op=OP.add)
                nc.sync.dma_start(out=out_d[b * P:b * P + bs, :], in_=outt[:bs, :])
    nc.compile()
    return nc




def _build_general(LT, flags):
    NCH = LT // P  # chunks per bucket
    nc = bacc.Bacc("TRN2", target_bir_lowering=False, detect_race_conditions=False)

    h_d = nc.dram_tensor("h", [N, D], F32, kind="ExternalInput")
    hl_d = nc.dram_tensor("hl", [NPAD, D], F32, kind="ExternalInput")
    dstrel_d = nc.dram_tensor("dstrel", [NB * LT], F32, kind="ExternalInput")
    srci_d = nc.dram_tensor("srci", [NB * LT], I32, kind="ExternalInput")
    refx_d = nc.dram_tensor("refx", [NB * LT, R + EF], F32, kind="ExternalInput")
    cd = {}
    cshapes = {
        "wdst": [D, 2 * D], "wsrc": [D, 2 * D], "wref": [R + EF, 2 * D + 1],
        "cb1": [1, 2 * D], "qw1": [D, D], "qb1": [1, D], "qw2": [D, D],
        "qb2": [1, D], "kw2": [D, D], "kb2": [1, D], "vw2": [D, D],
        "vb2": [1, D], "nw1a": [D, D], "nw1b": [D, D], "nb1": [1, D], "nw2": [D, D],
        "nb2": [1, D], "iotar": [P, P],
    }
    for nm in ("hk", "hv", "hq", "no"):
        if flags[nm + "_gb"]:
            cshapes[nm + "_grep"] = [P, D]
            cshapes[nm + "_brep"] = [P, D]
    for k, s in cshapes.items():
        cd[k] = nc.dram_tensor(k, s, F32, kind="ExternalInput")
    out_d = nc.dram_tensor("out", [NPC, D], F32, kind="ExternalOutput")

    qscale = 1.0 / np.sqrt(DH)

    with tile.TileContext(nc) as tc:
        with (
            tc.tile_pool(name="cpool", bufs=1) as cpool,
            tc.tile_pool(name="bpool", bufs=2) as bpool,
            tc.tile_pool(name="kpool", bufs=3) as kpool,
            tc.tile_pool(name="spool", bufs=4) as spool,
            tc.tile_pool(name="psum", bufs=1, space="PSUM") as ppool,
        ):
            # ---- constants resident in SBUF ----
            cs = {}
            for k, s in cshapes.items():
                t = cpool.tile(s, F32, tag="c_" + k)
                nc.sync.dma_start(out=t[:], in_=cd[k][:, :])
                cs[k] = t
            ident = cpool.tile([P, P], F32, tag="ident")
            make_identity(nc, ident[:])
            ones1 = cpool.tile([1, P], F32, tag="ones1")
            nc.vector.memset(ones1[:], 1.0)
            epsc = cpool.tile([P, 1], F32, tag="epsc")
            nc.vector.memset(epsc[:], EPS)

            def ln_relu(x_psum, out_sb, pref):
                """out_sb = relu(layernorm(x_psum) * g + beta), per-partition stats."""
                scr = spool.tile([P, P], F32, tag="scr")
                s1 = spool.tile([P, 1], F32, tag="s1")
                nc.scalar.activation(out=scr[:], in_=x_psum, func=AF.Copy,
                                     accum_out=s1[:])
                scr2 = spool.tile([P, P], F32, tag="scr2")
                s2 = spool.tile([P, 1], F32, tag="s2")
                nc.scalar.activation(out=scr2[:], in_=x_psum, func=AF.Square,
                                     accum_out=s2[:])
                mu = spool.tile([P, 1], F32, tag="mu")
                nc.vector.tensor_scalar_mul(mu[:], s1[:], 1.0 / D)
                var = spool.tile([P, 1], F32, tag="var")
                nc.vector.tensor_scalar(out=var[:], in0=s2[:], scalar1=1.0 / D,
                                        scalar2=None, op0=OP.mult)
                mu2 = spool.tile([P, 1], F32, tag="mu2")
                nc.vector.tensor_tensor(out=mu2[:], in0=mu[:], in1=mu[:], op=OP.mult)
                nc.vector.tensor_tensor(out=var[:], in0=var[:], in1=mu2[:],
                                        op=OP.subtract)
                sd = spool.tile([P, 1], F32, tag="sd")
                nc.scalar.activation(out=sd[:], in_=var[:], func=AF.Sqrt, bias=epsc[:])
                rs = spool.tile([P, 1], F32, tag="rs")
                nc.vector.reciprocal(rs[:], sd[:])
                nc.vector.tensor_scalar(out=out_sb, in0=x_psum, scalar1=mu[:],
                                        scalar2=rs[:], op0=OP.subtract, op1=OP.mult)
                if flags[pref + "_gb"]:
                    nc.vector.tensor_tensor(out=out_sb, in0=out_sb,
                                            in1=cs[pref + "_grep"][:], op=OP.mult)
                    nc.vector.tensor_tensor(out=out_sb, in0=out_sb,
                                            in1=cs[pref + "_brep"][:], op=OP.add)
                nc.vector.tensor_scalar_max(out_sb, out_sb, 0.0)

            def transpose_to_sb(src_sb, out_sb, np_, nf):
                """PE-transpose src_sb[:np_, :nf] -> out_sb[:nf, :np_] via PSUM."""
                tp = ppool.tile([P, P], F32, tag="tp", space="PSUM")
                nc.tensor.transpose(out=tp[:nf, :np_], in_=src_sb, identity=ident[:])
                nc.scalar.activation(out=out_sb, in_=tp[:nf, :np_], func=AF.Copy)

            for b in range(NB):
                bs = min(P, NPC - b * P)
                # ---------- bucket precompute ----------
                hlt = bpool.tile([P, D], F32, tag="hlt")
                nc.sync.dma_start(out=hlt[:], in_=hl_d[b * P:(b + 1) * P, :])
                hT = bpool.tile([P, P], F32, tag="hT")
                transpose_to_sb(hlt[:], hT[:], P, P)

                Bd = bpool.tile([P, 2 * D + 1 + D], F32, tag="Bd")  # [128, 385]

                # hW_dst = h_tile @ W1_dst (+ b1)  -> Bd[:, 0:256]
                hw_ps = ppool.tile([P, 2 * D], F32, tag="A", space="PSUM")
                nc.tensor.matmul(out=hw_ps[:], lhsT=hT[:], rhs=cs["wdst"][:],
                                 start=True, stop=not flags["cb1_nz"])
                if flags["cb1_nz"]:
                    nc.tensor.matmul(out=hw_ps[:], lhsT=ones1[:], rhs=cs["cb1"][:],
                                     start=False, stop=True)
                nc.scalar.activation(out=Bd[:, :2 * D], in_=hw_ps[:], func=AF.Copy)
                nc.vector.memset(Bd[:, 2 * D:2 * D + 1], flags["ew_b"])

                # q = MLP_q(h_tile) * qscale -> Bd[:, 257:385]
                q1_ps = ppool.tile([P, 2 * D], F32, tag="A", space="PSUM")
                nc.tensor.matmul(out=q1_ps[:, :D], lhsT=hT[:], rhs=cs["qw1"][:],
                                 start=True, stop=False)
                nc.tensor.matmul(out=q1_ps[:, :D], lhsT=ones1[:], rhs=cs["qb1"][:],
                                 start=False, stop=True)
                qz = bpool.tile([P, D], F32, tag="qz")
                ln_relu(q1_ps[:, :D], qz[:], "hq")
                qzT = bpool.tile([P, P], F32, tag="qzT")
                transpose_to_sb(qz[:], qzT[:], P, P)
                q2_ps = ppool.tile([P, 2 * D], F32, tag="A", space="PSUM")
                nc.tensor.matmul(out=q2_ps[:, :D], lhsT=qzT[:], rhs=cs["qw2"][:],
                                 start=True, stop=False)
                nc.tensor.matmul(out=q2_ps[:, :D], lhsT=ones1[:], rhs=cs["qb2"][:],
                                 start=False, stop=True)
                nc.scalar.activation(out=Bd[:, 2 * D + 1:], in_=q2_ps[:, :D],
                                     func=AF.Copy, scale=qscale)

                agg = ppool.tile([P, NH + D], F32, tag="agg", space="PSUM")

                # ---------- edge chunks ----------
                for ci in range(NCH):
                    e0 = b * LT + ci * P
                    dcol = kpool.tile([P, 1], F32, tag="dcol")
                    nc.sync.dma_start(out=dcol[:], in_=dstrelf_d[e0:e0 + P, None])
                    scol = kpool.tile([P, 1], I32, tag="scol")
                    nc.sync.dma_start(out=scol[:], in_=srci_d[e0:e0 + P, None])
                    refx = kpool.tile([P, R + EF], F32, tag="refx")
                    nc.sync.dma_start(out=refx[:], in_=refx_d[e0:e0 + P, :])
                    hsrc = kpool.tile([P, D], F32, tag="hsrc")
                    nc.gpsimd.indirect_dma_start(
                        out=hsrc[:], out_offset=None, in_=h_d[:, :],
                        in_offset=bass.IndirectOffsetOnAxis(ap=scol[:, :1], axis=0))

                    MT = kpool.tile([P, P], F32, tag="MT")
                    nc.vector.tensor_scalar(out=MT[:], in0=cs["iotar"][:],
                                            scalar1=dcol[:], scalar2=None,
                                            op0=OP.is_equal)
                    Mn = kpool.tile([P, P], F32, tag="Mn")
                    transpose_to_sb(MT[:], Mn[:], P, P)
                    hsT = kpool.tile([P, P], F32, tag="hsT")
                    transpose_to_sb(hsrc[:], hsT[:], P, P)
                    refT = kpool.tile([R + EF, P], F32, tag="refT")
                    transpose_to_sb(refx[:], refT[:], P, R + EF)

                    A = ppool.tile([P, 2 * D + 1 + D], F32, tag="A", space="PSUM")
                    nc.tensor.matmul(out=A[:], lhsT=Mn[:], rhs=Bd[:],
                                     start=True, stop=False)
                    nc.tensor.matmul(out=A[:, :2 * D], lhsT=hsT[:], rhs=cs["wsrc"][:],
                                     start=False, stop=False)
                    nc.tensor.matmul(out=A[:, :2 * D + 1], lhsT=refT[:],
                                     rhs=cs["wref"][:], start=False, stop=True)

                    zk = kpool.tile([P, D], F32, tag="zk")
                    ln_relu(A[:, :D], zk[:], "hk")
                    zv = kpool.tile([P, D], F32, tag="zv")
                    ln_relu(A[:, D:2 * D], zv[:], "hv")
                    zkT = kpool.tile([P, P], F32, tag="zkT")
                    transpose_to_sb(zk[:], zkT[:], P, P)
                    zvT = kpool.tile([P, P], F32, tag="zvT")
                    transpose_to_sb(zv[:], zvT[:], P, P)

                    k2 = ppool.tile([P, D], F32, tag="k2", space="PSUM")
                    nc.tensor.matmul(out=k2[:], lhsT=zkT[:], rhs=cs["kw2"][:],
                                     start=True, stop=not flags["kb2_nz"])
                    if flags["kb2_nz"]:
                        nc.tensor.matmul(out=k2[:], lhsT=ones1[:], rhs=cs["kb2"][:],
                                         start=False, stop=True)
                    v2 = ppool.tile([P, D], F32, tag="v2", space="PSUM")
                    nc.tensor.matmul(out=v2[:], lhsT=zvT[:], rhs=cs["vw2"][:],
                                     start=True, stop=not flags["vb2_nz"])
                    if flags["vb2_nz"]:
                        nc.tensor.matmul(out=v2[:], lhsT=ones1[:], rhs=cs["vb2"][:],
                                         start=False, stop=True)

                    ew = kpool.tile([P, 1], F32, tag="ew")
                    nc.scalar.activation(out=ew[:], in_=A[:, 2 * D:2 * D + 1],
                                         func=AF.Sigmoid)
                    k2s = kpool.tile([P, D], F32, tag="k2s")
                    nc.scalar.activation(out=k2s[:], in_=k2[:], func=AF.Copy)
                    lg = kpool.tile([P, D], F32, tag="lg")
                    nc.vector.tensor_tensor(out=lg[:], in0=A[:, 2 * D + 1:],
                                            in1=k2s[:], op=OP.mult)
                    lgh = kpool.tile([P, NH], F32, tag="lgh")
                    nc.vector.tensor_reduce(
                        out=lgh[:], in_=lg[:].rearrange("p (h d) -> p h d", d=DH),
                        axis=mybir.AxisListType.X, op=OP.add)

                    exm = kpool.tile([P, NH + D], F32, tag="exm")
                    nc.scalar.activation(out=exm[:, :NH], in_=lgh[:], func=AF.Exp)
                    vw = kpool.tile([P, D], F32, tag="vw")
                    nc.vector.tensor_scalar_mul(vw[:], v2[:], ew[:])
                    nc.vector.tensor_tensor(
                        out=exm[:, NH:].rearrange("p (h d) -> p h d", d=DH),
                        in0=vw[:].rearrange("p (h d) -> p h d", d=DH),
                        in1=exm[:, :NH][:, :, None].to_broadcast([P, NH, DH]),
                        op=OP.mult)

                    nc.tensor.matmul(out=agg[:], lhsT=MT[:], rhs=exm[:],
                                     start=(ci == 0), stop=(ci == NCH - 1),
                                     skip_group_check=True)

                # ---------- bucket epilogue ----------
                den = bpool.tile([P, NH], F32, tag="den")
                nc.vector.tensor_scalar_max(den[:], agg[:, :NH], 1e-30)
                rd = bpool.tile([P, NH], F32, tag="rd")
                nc.vector.reciprocal(rd[:], den[:])
                attn = bpool.tile([P, D], F32, tag="attn")
                nc.vector.tensor_tensor(
                    out=attn[:].rearrange("p (h d) -> p h d", d=DH),
                    in0=agg[:, NH:].rearrange("p (h d) -> p h d", d=DH),
                    in1=rd[:][:, :, None].to_broadcast([P, NH, DH]),
                    op=OP.mult)
                attnT = bpool.tile([P, P], F32, tag="attnT")
                transpose_to_sb(attn[:], attnT[:], P, P)

                f1_ps = ppool.tile([P, 2 * D], F32, tag="A", space="PSUM")
                nc.tensor.matmul(out=f1_ps[:, :D], lhsT=attnT[:], rhs=cs["nw1a"][:],
                                 start=True, stop=False)
                nc.tensor.matmul(out=f1_ps[:, :D], lhsT=hT[:], rhs=cs["nw1b"][:],
                                 start=False, stop=False)
                nc.tensor.matmul(out=f1_ps[:, :D], lhsT=ones1[:], rhs=cs["nb1"][:],
                                 start=False, stop=True)
                fz = bpool.tile([P, D], F32, tag="fz")
                ln_relu(f1_ps[:, :D], fz[:], "no")
                fzT = bpool.tile([P, P], F32, tag="fzT")
                transpose_to_sb(fz[:], fzT[:], P, P)
                f2_ps = ppool.tile([P, 2 * D], F32, tag="A", space="PSUM")
                nc.tensor.matmul(out=f2_ps[:, :D], lhsT=fzT[:], rhs=cs["nw2"][:],
                                 start=True, stop=False)
                nc.tensor.matmul(out=f2_ps[:, :D], lhsT=ones1[:], rhs=cs["nb2"][:],
                                 start=False, stop=True)
                outt = bpool.tile([P, D], F32, tag="outt")
                nc.vector.tensor_tensor(out=outt[:], in0=f2_ps[:, :D], in1=hlt[:],
                                        op=OP.add)
                nc.sync.dma_start(out=out_d[b * P:b * P + bs, :], in_=outt[:bs, :])
    nc.compile()
    return nc


BF16 = mybir.dt.bfloat16


